# revision 1
# baseline (speedup 1.0000x reference)
"""BotRGCN + MoE (top-1 of 2) Trainium2 Bass kernel, 8-core SPMD. v2.

Key points vs v1:
  - fp32 throughout the x-pipeline (MoE top-1 gate sign is precision-critical:
    fp16/f32r tables flip experts for borderline nodes -> O(1) error).
  - Gather table split into AG_CHUNKS chunk tensors (per-chunk AllGather,
    emitted as soon as the producing windows are exported -> overlaps wire
    time with compute). Chunk row spaces stay < 32768 so int16 indexing works
    without the old lo/hi split.
  - MoE expert weights/h1 in float32r (post-gate path only; 4x PE speedup on
    those matmuls, y-error ~1e-3 rel, gate unaffected).
  - cnt_inv sent as [*, 1, WIN] and broadcast on-device via a rank-1 matmul.

Self-contained: hardcodes shapes; imports only installed packages.
"""

import numpy as np

N = 50000
E = 400000
D = 256
R = 2
NE = 2
OUT = 256
NCORES = 8
NLOC = N // NCORES  # 6250
WIN = 512
NWIN = (NLOC + WIN - 1) // WIN  # 13
CHUNK = 128
N_GRID = 128

# config
AG_CHUNKS = 2         # table chunk tensors / collectives per layer (>=2)
AG_OVERLAP = True     # emit chunk AllGathers inline with producing windows
MOE_F32R = True       # MoE expert matmuls in float32r
N_SWDGE_Q = 2         # swdge queues for gathers
HILO = True           # table as fp16 hi + fp16 residual (exact to ~1e-7);
                      # selector matmuls run fp16 at 1 cyc/col
REPS = 1              # repeat whole body (marginal-cost timing)
SKIP_COLL = False     # timeline-sim only: skip collectives

SELU_SCALE = 1.0507009873554805
SELU_ALPHA = 1.6732632423543772
NEG_SLOPE = 0.01


def _chunk_bounds(agc):
    # WIN-aligned, slightly unbalanced splits minimize per-group ceil padding
    table = {
        2: [0, 3584, NLOC],
        3: [0, 2048, 4096, NLOC],
        4: [0, 1536, 3072, 4608, NLOC],
    }
    if agc in table:
        return table[agc]
    base = max(WIN, (NLOC // agc) // WIN * WIN)
    return [min(k * base, NLOC) for k in range(agc)] + [NLOC]


def _wrap_idx(idx):
    """int16 index list (len multiple of 16) -> (128, len/16) wrapped+replicated."""
    n = len(idx)
    w = idx.reshape(n // 16, 16).T.astype(np.int16)
    return np.tile(w, (8, 1))


# ----------------------------------------------------------------------------
# host-side planning
# ----------------------------------------------------------------------------

def build_plan(edge_index, edge_type):
    agc = AG_CHUNKS
    bounds = _chunk_bounds(agc)
    csz = [bounds[k + 1] - bounds[k] for k in range(agc)]

    src = np.asarray(edge_index[0], dtype=np.int64)
    dst = np.asarray(edge_index[1], dtype=np.int64)
    rel = np.asarray(edge_type, dtype=np.int64)

    core = dst // NLOC
    seg = rel * NLOC + (dst % NLOC)
    NSEG = R * NLOC

    cnt = np.bincount((core * NSEG + seg).astype(np.int64),
                      minlength=NCORES * NSEG).reshape(NCORES, NSEG)

    # chunk + row-in-chunk-tensor of each edge's source node
    src_c = src // NLOC
    src_j = src % NLOC
    src_k = np.searchsorted(bounds, src_j, side="right") - 1
    csz_arr = np.asarray(csz)
    b_arr = np.asarray(bounds[:-1])
    src_row = src_c * csz_arr[src_k] + (src_j - b_arr[src_k])

    # groups: (r, wbase, gbase, gsize, win_index)
    groups = []
    wi = 0
    for r in range(R):
        for w in range(NWIN):
            wb = w * WIN
            nw = min(WIN, NLOC - wb)
            g0 = 0
            while g0 < nw:
                gs = min(N_GRID, nw - g0)
                groups.append((r, wb, wb + g0, gs, wi))
                g0 += gs
            wi += 1
    n_windows_total = wi
    NG = len(groups)

    win_groups = [[] for _ in range(n_windows_total)]
    for gi, g in enumerate(groups):
        win_groups[g[4]].append(gi)

    # per-core edge lists sorted by (seg, chunk)
    per_core = []
    for c in range(NCORES):
        m = core == c
        s_seg, s_row, s_k = seg[m], src_row[m], src_k[m]
        o = np.lexsort((s_k, s_seg))
        per_core.append((s_seg[o], s_row[o], s_k[o]))

    # slots per (group, chunk): max over cores of ceil(count/128)
    n_gk = np.zeros((NG, agc), np.int64)
    core_group_edges = []  # [core][gi][k] -> (rows, seg_local)
    for c in range(NCORES):
        s_seg, s_row, s_k = per_core[c]
        lst = []
        for gi, (r, wb, gb, gs, _) in enumerate(groups):
            lo_b = np.searchsorted(s_seg, r * NLOC + gb)
            hi_b = np.searchsorted(s_seg, r * NLOC + gb + gs)
            rows, gg, kk = (s_row[lo_b:hi_b], s_seg[lo_b:hi_b] - (r * NLOC + gb),
                            s_k[lo_b:hi_b])
            per_k = []
            for k in range(agc):
                mk = kk == k
                per_k.append((rows[mk], gg[mk]))
                n_gk[gi, k] = max(n_gk[gi, k], -(-int(mk.sum()) // CHUNK))
            lst.append(per_k)
        core_group_edges.append(lst)

    # every (win, rel) needs >=1 slot so the PSUM bank gets cleared
    for w in range(n_windows_total):
        if sum(int(n_gk[gi, k]) for gi in win_groups[w] for k in range(agc)) == 0:
            n_gk[win_groups[w][0], 0] = 1

    # slot layout per (win, rel): chunk-major, then group order
    slot_group = []
    win_slot_chunks = []  # per win: list over k of slot count
    win_slot_start = []
    for w in range(n_windows_total):
        start = len(slot_group)
        per_k_counts = []
        for k in range(agc):
            nk = 0
            for gi in win_groups[w]:
                slot_group.extend([gi] * int(n_gk[gi, k]))
                nk += int(n_gk[gi, k])
            per_k_counts.append(nk)
        win_slot_start.append(start)
        win_slot_chunks.append(per_k_counts)
    n_slots = len(slot_group)

    idx_all = np.zeros((NCORES, n_slots, CHUNK), np.int16)
    seg_all = np.full((NCORES, CHUNK, n_slots), -1.0, np.float32)
    for c in range(NCORES):
        for w in range(n_windows_total):
            cursor = win_slot_start[w]
            for k in range(agc):
                for gi in win_groups[w]:
                    nsl = int(n_gk[gi, k])
                    if nsl == 0:
                        continue
                    rows, gg = core_group_edges[c][gi][k]
                    ne = len(rows)
                    pad = nsl * CHUNK - ne
                    rr = np.concatenate([rows, np.zeros(pad, np.int64)])
                    gp = np.concatenate([gg, np.full(pad, -1, np.int64)])
                    for s in range(nsl):
                        sl = cursor + s
                        idx_all[c, sl] = rr[s * CHUNK:(s + 1) * CHUNK].astype(np.int16)
                        seg_all[c, :, sl] = gp[s * CHUNK:(s + 1) * CHUNK].astype(np.float32)
                    cursor += nsl

    # idx16 column layout: per (win, rel), per chunk
    win_idx_cols = []  # per win: [(col, ni), ...] per chunk
    col = 0
    for w in range(n_windows_total):
        entry = []
        for k in range(agc):
            ni = win_slot_chunks[w][k] * CHUNK
            entry.append((col, ni))
            col += ni // 16
        win_idx_cols.append(entry)
    tot_cols = col

    idx16 = []
    for c in range(NCORES):
        buf = np.zeros((128, tot_cols), np.int16)
        for w in range(n_windows_total):
            cursor = win_slot_start[w]
            for k in range(agc):
                nsl = win_slot_chunks[w][k]
                ck, ni = win_idx_cols[w][k]
                if ni:
                    buf[:, ck:ck + ni // 16] = _wrap_idx(
                        idx_all[c, cursor:cursor + nsl].reshape(-1))
                cursor += nsl
        idx16.append(buf)

    cntinv = np.ones((NCORES, n_windows_total, 1, WIN), np.float32)
    for c in range(NCORES):
        for r in range(R):
            for w in range(NWIN):
                wb = w * WIN
                nw = min(WIN, NLOC - wb)
                cc = cnt[c, r * NLOC + wb: r * NLOC + wb + nw]
                cntinv[c, r * NWIN + w, 0, :nw] = 1.0 / np.maximum(cc, 1)

    plan = dict(
        agc=agc, bounds=bounds, csz=csz,
        groups=groups,
        slot_group=np.array(slot_group, np.int64),
        win_groups=win_groups,
        win_slot_start=win_slot_start,
        win_slot_chunks=win_slot_chunks,
        win_idx_cols=win_idx_cols,
        n_slots=n_slots,
        idx_cols=tot_cols,
        n_windows_total=n_windows_total,
        max_slots=max(sum(cc) for cc in win_slot_chunks),
    )
    data = dict(idx16=idx16, seg_all=seg_all, cntinv=cntinv)
    return plan, data


# ----------------------------------------------------------------------------
# bass program
# ----------------------------------------------------------------------------

def build_nc(plan):
    import concourse.mybir as mybir
    import concourse.tile as tile
    from concourse import bacc
    from concourse.masks import make_identity

    dt = mybir.dt
    f32 = dt.float32
    f32r = dt.float32r
    edt = f32r if MOE_F32R else f32   # MoE expert dtype
    Alu = mybir.AluOpType
    ACT = mybir.ActivationFunctionType

    agc = plan["agc"]
    bounds = plan["bounds"]
    csz = plan["csz"]
    NSEGW = plan["n_windows_total"]
    NSLOT = plan["n_slots"]
    IDXC = plan["idx_cols"]
    MAX_SLOTS = plan["max_slots"]
    groups = plan["groups"]
    slot_group = plan["slot_group"]

    nc = bacc.Bacc(None, num_devices=NCORES, num_swdge_queues=N_SWDGE_Q)

    xcatT_in = nc.dram_tensor("xcatT", [128, 2, NLOC], f32, kind="ExternalInput")
    idx16_in = nc.dram_tensor("idx16", [128, IDXC], dt.int16, kind="ExternalInput")
    seg_in = nc.dram_tensor("segloc", [128, NSLOT], f32, kind="ExternalInput")
    cntinv_in = nc.dram_tensor("cntinv", [NSEGW, 1, WIN], f32, kind="ExternalInput")
    w_in_in = nc.dram_tensor("w_in", [128, 2, D], f32, kind="ExternalInput")
    w_root_in = nc.dram_tensor("w_root", [128, 2, D], f32, kind="ExternalInput")
    w_rel_in = nc.dram_tensor("w_rel", [R, 128, 2, D], f32, kind="ExternalInput")
    b_in_in = nc.dram_tensor("b_in", [128, 2], f32, kind="ExternalInput")
    b_rgcn_in = nc.dram_tensor("b_rgcn", [128, 2], f32, kind="ExternalInput")
    wg_in = nc.dram_tensor("wgate", [128, 2, 1], f32, kind="ExternalInput")
    we1_in = nc.dram_tensor("we1", [NE, 128, 2, D], edt, kind="ExternalInput")
    be1_in = nc.dram_tensor("be1", [NE, 128, 2], f32, kind="ExternalInput")
    we2_in = nc.dram_tensor("we2", [NE, 128, 2, OUT], edt, kind="ExternalInput")
    be2_in = nc.dram_tensor("be2row", [1, NE, 2, 128], f32, kind="ExternalInput")
    out_t = nc.dram_tensor("out", [OUT, NLOC], f32, kind="ExternalOutput")

    with tile.TileContext(nc) as tc:
        with (
            tc.tile_pool(name="const", bufs=1) as cpool,
            tc.tile_pool(name="work", bufs=2) as wpool,
            tc.tile_pool(name="slabp", bufs=3) as slabpool,
            tc.tile_pool(name="selp", bufs=4) as selpool,
            tc.tile_pool(name="stage", bufs=3) as stpool,
            tc.tile_pool(name="psum_sel", bufs=2, space="PSUM") as ps_sel,
            tc.tile_pool(name="psum_xf", bufs=2, space="PSUM") as ps_xf,
            tc.tile_pool(name="psum_misc", bufs=2, space="PSUM") as ps_misc,
            tc.tile_pool(name="dram", bufs=1, space="DRAM") as dpool,
            tc.tile_pool(name="dramsh", bufs=1, space="DRAM") as shpool,
        ):
            # constants / weights
            ident = cpool.tile([128, 128], f32)
            make_identity(nc, ident[:])
            iota_i = cpool.tile([128, N_GRID], dt.int32)
            nc.gpsimd.iota(iota_i[:], pattern=[[1, N_GRID]], base=0,
                           channel_multiplier=0)
            iota_f = cpool.tile([128, N_GRID], f32)
            nc.vector.tensor_copy(iota_f[:], iota_i[:])
            ones_row = cpool.tile([1, 128], f32)
            nc.vector.memset(ones_row[:], 1.0)

            def load_const(t_in, shape, re=None, tag=None, cdt=f32):
                t = cpool.tile(shape, cdt, tag=tag)
                nc.sync.dma_start(t[:], t_in[:] if re is None else t_in[:].rearrange(re))
                return t

            w_in_sb = load_const(w_in_in, [128, 2, D], tag="w_in")
            w_root_sb = load_const(w_root_in, [128, 2, D], tag="w_root")
            w_rel_sb = load_const(w_rel_in, [128, R, 2, D], "r p k d -> p r k d",
                                  tag="w_rel")
            b_in_sb = load_const(b_in_in, [128, 2], tag="b_in")
            b_rg_sb = load_const(b_rgcn_in, [128, 2], tag="b_rg")
            wgd_sb = load_const(wg_in, [128, 2, 1], tag="wgd")
            we1_sb = load_const(we1_in, [128, NE, 2, D], "e p k d -> p e k d",
                                tag="we1", cdt=edt)
            be1_sb = load_const(be1_in, [128, NE, 2], "e p k -> p e k", tag="be1")
            we2_sb = load_const(we2_in, [128, NE, 2, OUT], "e p k d -> p e k d",
                                tag="we2", cdt=edt)
            be2_sb = cpool.tile([1, NE, 2, 128], f32)
            nc.sync.dma_start(be2_sb[:], be2_in[:])

            seg_sb = cpool.tile([128, NSLOT], f32)
            nc.sync.dma_start(seg_sb[:], seg_in[:])
            idx_sb = cpool.tile([128, IDXC], dt.int16)
            nc.sync.dma_start(idx_sb[:], idx16_in[:])

            # DRAM staging
            xT1 = dpool.tile([128, 2, NLOC], f32)
            xT2 = dpool.tile([128, 2, NLOC], f32)
            tdt = dt.float16 if HILO else f32
            TW = 2 * D if HILO else D   # table row width in tdt elements
            xloc1 = dpool.tile([NLOC, TW], tdt)
            xloc2 = dpool.tile([NLOC, TW], tdt)
            xfulls = []  # [rep][layer][chunk]
            for _r in range(REPS):
                per_layer = []
                for li in (1, 2):
                    per_layer.append([
                        shpool.tile([NCORES * csz[k], TW], tdt,
                                    addr_space="Shared",
                                    name="xf%d_%d_%d" % (li, _r, k))
                        for k in range(agc)
                    ])
                xfulls.append(per_layer)

            def win_sizes(w):
                wb = w * WIN
                return wb, min(WIN, NLOC - wb)

            def load_xwin(src_dram, wb, nw, tag):
                t = wpool.tile([128, 2, WIN], f32, tag=tag)
                nc.sync.dma_start(t[:, :, :nw], src_dram[:, :, wb:wb + nw])
                return t

            def export_window(xw, wb, nw, xloc):
                # transpose (128, 2, nw) -> node-major rows of xloc
                nb = 0
                while nb < nw:
                    bs = min(128, nw - nb)
                    stg = stpool.tile([128, TW], tdt, tag="stage")
                    for mc in range(2):
                        pst = ps_misc.tile([128, max(WIN, 128)], f32,
                                           space="PSUM", tag="misc")
                        nc.tensor.transpose(pst[:bs, :128], xw[:, mc, nb:nb + bs],
                                            ident[:])
                        nc.scalar.activation(stg[:bs, mc * 128:(mc + 1) * 128],
                                             pst[:bs, :128], ACT.Copy)
                        if HILO:
                            # residual lo = x - float(hi), stored fp16
                            tmp = wpool.tile([128, 128], f32, tag="hilo_tmp")
                            nc.vector.tensor_tensor(
                                out=tmp[:bs, :], in0=pst[:bs, :128],
                                in1=stg[:bs, mc * 128:(mc + 1) * 128],
                                op=Alu.subtract)
                            nc.scalar.activation(
                                stg[:bs, D + mc * 128:D + (mc + 1) * 128],
                                tmp[:bs, :], ACT.Copy)
                    nc.sync.dma_start(xloc[wb + nb: wb + nb + bs, :], stg[:bs, :])
                    nb += bs

            for _rep in range(REPS):
                xfull1, xfull2 = xfulls[_rep]
                ag_state1, ag_state2 = [0], [0]

                def ag_emit(xloc, xflist, rows_done, state):
                    while state[0] < agc and bounds[state[0] + 1] <= rows_done:
                        k = state[0]
                        if not SKIP_COLL:
                            nc.gpsimd.collective_compute(
                                "AllGather", mybir.AluOpType.bypass,
                                replica_groups=[list(range(NCORES))],
                                ins=[xloc[bounds[k]:bounds[k + 1], :].opt()],
                                outs=[xflist[k][:].opt()])
                        state[0] += 1

                # ------------ layer 0: x1 = selu(x_cat @ W_in + b_in) --------
                for w in range(NWIN):
                    wb, nw = win_sizes(w)
                    xw = load_xwin(xcatT_in, wb, nw, "xw")
                    xo = wpool.tile([128, 2, WIN], f32, tag="xo")
                    for mc in range(2):
                        ps = ps_xf.tile([128, WIN], f32, space="PSUM", tag="xf")
                        for kc in range(2):
                            nc.tensor.matmul(
                                ps[:, :nw],
                                w_in_sb[:, kc, mc * 128:(mc + 1) * 128],
                                xw[:, kc, :nw],
                                start=(kc == 0), stop=(kc == 1),
                            )
                        pos = wpool.tile([128, WIN], f32, tag="selu_pos")
                        nc.vector.tensor_scalar(
                            out=pos[:, :nw], in0=ps[:, :nw],
                            scalar1=b_in_sb[:, mc:mc + 1], scalar2=0.0,
                            op0=Alu.add, op1=Alu.max)
                        neg = wpool.tile([128, WIN], f32, tag="selu_neg")
                        nc.vector.tensor_scalar(
                            out=neg[:, :nw], in0=ps[:, :nw],
                            scalar1=b_in_sb[:, mc:mc + 1], scalar2=0.0,
                            op0=Alu.add, op1=Alu.min)
                        e = wpool.tile([128, WIN], f32, tag="selu_e")
                        nc.scalar.activation(e[:, :nw], neg[:, :nw], ACT.Exp)
                        sa = SELU_SCALE * SELU_ALPHA
                        nc.vector.tensor_scalar(
                            out=e[:, :nw], in0=e[:, :nw], scalar1=sa, scalar2=sa,
                            op0=Alu.mult, op1=Alu.subtract)
                        nc.vector.tensor_scalar(
                            out=pos[:, :nw], in0=pos[:, :nw],
                            scalar1=SELU_SCALE, scalar2=None, op0=Alu.mult)
                        nc.vector.tensor_tensor(
                            out=xo[:, mc, :nw], in0=pos[:, :nw], in1=e[:, :nw],
                            op=Alu.add)
                    nc.sync.dma_start(xT1[:, :, wb:wb + nw], xo[:, :, :nw])
                    export_window(xo, wb, nw, xloc1)
                    if AG_OVERLAP:
                        ag_emit(xloc1, xfull1, wb + nw, ag_state1)
                ag_emit(xloc1, xfull1, NLOC, ag_state1)

                # ------------ rgcn layers ------------
                def rgcn_layer(xfull, xT_cur, xT_next, xloc_next, li,
                               moe_fn=None, xfull_next=None, ag_state=None):
                    for w in range(NWIN):
                        wb, nw = win_sizes(w)
                        s_tiles = {}
                        for r in range(R):
                            wi = r * NWIN + w
                            cursor = plan["win_slot_start"][wi]
                            kcounts = plan["win_slot_chunks"][wi]
                            nslots_w = sum(kcounts)
                            slab = slabpool.tile([128, MAX_SLOTS, TW], tdt,
                                                 tag="slab")
                            off = 0
                            for k in range(agc):
                                ck, ni = plan["win_idx_cols"][wi][k]
                                nsl = kcounts[k]
                                if ni:
                                    nc.gpsimd.dma_gather(
                                        out_ap=slab[:, off:off + nsl, :],
                                        in_ap=xfull[k][:],
                                        idxs_ap=idx_sb[:, ck:ck + ni // 16],
                                        num_idxs=ni, num_idxs_reg=ni,
                                        elem_size=TW, single_packet=False,
                                        queue_num=(r * agc + k) % N_SWDGE_Q)
                                off += nsl
                            ps0 = ps_sel.tile([128, WIN], f32, space="PSUM",
                                              tag="sel0")
                            ps1 = ps_sel.tile([128, WIN], f32, space="PSUM",
                                              tag="sel1")
                            for s in range(nslots_w):
                                sl = cursor + s
                                gi = int(slot_group[sl])
                                gb_in_win = groups[gi][2] - wb
                                gs = groups[gi][3]
                                sel = selpool.tile([128, N_GRID], tdt, tag="sel")
                                nc.vector.tensor_scalar(
                                    out=sel[:, :gs], in0=iota_f[:, :gs],
                                    scalar1=seg_sb[:, sl:sl + 1], scalar2=None,
                                    op0=Alu.is_equal)
                                cols = slice(gb_in_win, gb_in_win + gs)
                                last = s == nslots_w - 1
                                if HILO:
                                    nc.tensor.matmul(
                                        ps0[:, cols], slab[:, s, 0:128],
                                        sel[:, :gs], start=(s == 0), stop=False)
                                    nc.tensor.matmul(
                                        ps0[:, cols], slab[:, s, 256:384],
                                        sel[:, :gs], start=False, stop=last)
                                    nc.tensor.matmul(
                                        ps1[:, cols], slab[:, s, 128:256],
                                        sel[:, :gs], start=(s == 0), stop=False)
                                    nc.tensor.matmul(
                                        ps1[:, cols], slab[:, s, 384:512],
                                        sel[:, :gs], start=False, stop=last)
                                else:
                                    nc.tensor.matmul(
                                        ps0[:, cols], slab[:, s, 0:128],
                                        sel[:, :gs], start=(s == 0), stop=last)
                                    nc.tensor.matmul(
                                        ps1[:, cols], slab[:, s, 128:256],
                                        sel[:, :gs], start=(s == 0), stop=last)
                            ci_row = wpool.tile([1, WIN], f32, tag="ci_row")
                            nc.sync.dma_start(ci_row[:], cntinv_in[wi])
                            cib = ps_misc.tile([128, WIN], f32, space="PSUM",
                                               tag="misc")
                            nc.tensor.matmul(cib[:, :nw], ones_row[:],
                                             ci_row[:, :nw], start=True, stop=True)
                            ci = wpool.tile([128, WIN], f32, tag="cntinv")
                            nc.scalar.activation(ci[:, :nw], cib[:, :nw], ACT.Copy)
                            s0 = wpool.tile([128, WIN], f32, tag="s0_%d" % r)
                            s1 = wpool.tile([128, WIN], f32, tag="s1_%d" % r)
                            nc.vector.tensor_tensor(out=s0[:, :nw], in0=ps0[:, :nw],
                                                    in1=ci[:, :nw], op=Alu.mult)
                            nc.vector.tensor_tensor(out=s1[:, :nw], in0=ps1[:, :nw],
                                                    in1=ci[:, :nw], op=Alu.mult)
                            s_tiles[r] = (s0, s1)

                        xw = load_xwin(xT_cur, wb, nw, "xw")
                        xo = wpool.tile([128, 2, WIN], f32, tag="xo")
                        for mc in range(2):
                            ps = ps_xf.tile([128, WIN], f32, space="PSUM", tag="xf")
                            for kc in range(2):
                                nc.tensor.matmul(
                                    ps[:, :nw],
                                    w_root_sb[:, kc, mc * 128:(mc + 1) * 128],
                                    xw[:, kc, :nw],
                                    start=(kc == 0), stop=False)
                            for r in range(R):
                                for kc in range(2):
                                    st = s_tiles[r][kc]
                                    nc.tensor.matmul(
                                        ps[:, :nw],
                                        w_rel_sb[:, r, kc, mc * 128:(mc + 1) * 128],
                                        st[:, :nw],
                                        start=False, stop=(r == R - 1 and kc == 1))
                            nc.vector.tensor_scalar(
                                out=xo[:, mc, :nw], in0=ps[:, :nw],
                                scalar1=b_rg_sb[:, mc:mc + 1], scalar2=None,
                                op0=Alu.add)
                        if xT_next is not None:
                            nc.sync.dma_start(xT_next[:, :, wb:wb + nw],
                                              xo[:, :, :nw])
                        if xloc_next is not None:
                            export_window(xo, wb, nw, xloc_next)
                            if xfull_next is not None and AG_OVERLAP:
                                ag_emit(xloc_next, xfull_next, wb + nw, ag_state)
                        if moe_fn is not None:
                            moe_fn(xo, wb, nw)
                    if xfull_next is not None:
                        ag_emit(xloc_next, xfull_next, NLOC, ag_state)

                # ------------ MoE (fused into layer-2 windows) ------------
                def moe_window(xw, wb, nw):
                    psl = ps_misc.tile([128, WIN], f32, space="PSUM", tag="misc")
                    for kc in range(2):
                        nc.tensor.matmul(
                            psl[:1, :nw], wgd_sb[:, kc, :], xw[:, kc, :nw],
                            start=(kc == 0), stop=(kc == 1))
                    g_row = wpool.tile([1, WIN], f32, tag="grow")
                    nc.vector.tensor_scalar(out=g_row[:, :nw], in0=psl[:1, :nw],
                                            scalar1=0.0, scalar2=None,
                                            op0=Alu.is_ge)
                    ginv_row = wpool.tile([1, WIN], f32, tag="ginvrow")
                    nc.vector.tensor_scalar(out=ginv_row[:, :nw], in0=g_row[:, :nw],
                                            scalar1=-1.0, scalar2=1.0,
                                            op0=Alu.mult, op1=Alu.add)
                    psb = ps_misc.tile([128, WIN], f32, space="PSUM", tag="misc")
                    nc.tensor.matmul(psb[:, :nw], ones_row[:], g_row[:, :nw],
                                     start=True, stop=True)
                    gb = wpool.tile([128, WIN], f32, tag="gb_sb")
                    nc.scalar.activation(gb[:, :nw], psb[:, :nw], ACT.Copy)
                    ginv = wpool.tile([128, WIN], f32, tag="ginv")
                    nc.vector.tensor_scalar(out=ginv[:, :nw], in0=gb[:, :nw],
                                            scalar1=-1.0, scalar2=1.0,
                                            op0=Alu.mult, op1=Alu.add)

                    if MOE_F32R:
                        xwr = wpool.tile([128, 2, WIN], edt, tag="xwr")
                        for kc in range(2):
                            nc.scalar.activation(xwr[:, kc, :nw], xw[:, kc, :nw],
                                                 ACT.Copy)
                    else:
                        xwr = xw

                    h1g = {}
                    for e in range(NE):
                        for mc in range(2):
                            psh = ps_xf.tile([128, WIN], f32, space="PSUM",
                                             tag="xf")
                            for kc in range(2):
                                nc.tensor.matmul(
                                    psh[:, :nw],
                                    we1_sb[:, e, kc, mc * 128:(mc + 1) * 128],
                                    xwr[:, kc, :nw],
                                    start=(kc == 0), stop=(kc == 1))
                            h = wpool.tile([128, WIN], edt,
                                           tag="h1_%d_%d" % (e, mc))
                            nc.scalar.activation(
                                h[:, :nw], psh[:, :nw], ACT.Lrelu,
                                bias=be1_sb[:, e, mc:mc + 1], alpha=NEG_SLOPE)
                            gt = gb if e == 0 else ginv
                            nc.vector.tensor_tensor(out=h[:, :nw], in0=h[:, :nw],
                                                    in1=gt[:, :nw], op=Alu.mult)
                            h1g[(e, mc)] = h
                    for mc in range(2):
                        psy = ps_xf.tile([128, WIN], f32, space="PSUM", tag="xf")
                        first = True
                        for e in range(NE):
                            for kc in range(2):
                                nc.tensor.matmul(
                                    psy[:, :nw],
                                    we2_sb[:, e, kc, mc * 128:(mc + 1) * 128],
                                    h1g[(e, kc)][:, :nw],
                                    start=first, stop=False)
                                first = False
                        nc.tensor.matmul(psy[:, :nw], be2_sb[0:1, 0, mc, :],
                                         g_row[:, :nw], start=False, stop=False)
                        nc.tensor.matmul(psy[:, :nw], be2_sb[0:1, 1, mc, :],
                                         ginv_row[:, :nw], start=False, stop=True)
                        yt = wpool.tile([128, WIN], f32, tag="yt")
                        nc.scalar.activation(yt[:, :nw], psy[:, :nw], ACT.Copy)
                        nc.sync.dma_start(out_t[mc * 128:(mc + 1) * 128,
                                                wb:wb + nw],
                                          yt[:, :nw])

                rgcn_layer(xfull1, xT1, xT2, xloc2, 1,
                           xfull_next=xfull2, ag_state=ag_state2)
                rgcn_layer(xfull2, xT2, None, None, 2, moe_fn=moe_window)

    nc.compile()
    return nc


# ----------------------------------------------------------------------------
# entry point
# ----------------------------------------------------------------------------

def kernel(des, tweet, num_prop, cat_prop, edge_index, edge_type,
           W_in, b_in, W_rel, W_root, b_rgcn, w_gate, We1, be1, We2, be2):
    from concourse.bass_utils import run_bass_kernel_spmd

    x_cat = np.concatenate(
        [np.asarray(des), np.asarray(tweet), np.asarray(num_prop),
         np.asarray(cat_prop)], axis=1).astype(np.float32)

    plan, data = build_plan(np.asarray(edge_index), np.asarray(edge_type))
    nc = build_nc(plan)

    def wmat(w):
        w = np.asarray(w, np.float32)
        return np.ascontiguousarray(w.reshape(2, 128, w.shape[1]).transpose(1, 0, 2))

    def bvec(b):
        return np.ascontiguousarray(np.asarray(b, np.float32).reshape(2, 128).T)

    w_rel_h = np.stack([wmat(np.asarray(W_rel)[r]) for r in range(R)])
    we1_h = np.stack([wmat(np.asarray(We1)[e]) for e in range(NE)])
    be1_h = np.stack([bvec(np.asarray(be1)[e]) for e in range(NE)])
    we2_h = np.stack([wmat(np.asarray(We2)[e]) for e in range(NE)])
    be2row = np.asarray(be2, np.float32).reshape(1, NE, 2, 128)

    in_maps = []
    for c in range(NCORES):
        xc = x_cat[c * NLOC:(c + 1) * NLOC]
        xcatT = np.ascontiguousarray(xc.T.reshape(2, 128, NLOC).transpose(1, 0, 2))
        in_maps.append({
            "xcatT": xcatT,
            "idx16": data["idx16"][c],
            "segloc": data["seg_all"][c],
            "cntinv": data["cntinv"][c],
            "w_in": wmat(W_in), "w_root": wmat(W_root), "w_rel": w_rel_h,
            "b_in": bvec(b_in), "b_rgcn": bvec(b_rgcn),
            "wgate": wmat(np.asarray(w_gate)[:, 0:1] - np.asarray(w_gate)[:, 1:2]),
            "we1": we1_h, "be1": be1_h,
            "we2": we2_h, "be2row": be2row,
        })

    res = run_bass_kernel_spmd(nc, in_maps, core_ids=list(range(NCORES)))
    global last_nc, last_in_maps
    last_nc, last_in_maps = nc, in_maps
    y = np.concatenate([res.results[c]["out"].T for c in range(NCORES)], axis=0)
    return y.astype(np.float32)


last_nc = None
last_in_maps = None



# revision 12
# speedup vs baseline: 50.0817x; 50.0817x over previous
"""BotRGCN + MoE (top-1 of 2) Trainium2 Bass kernel, 8-core SPMD. v3.

v3 over v2 (PE was the bottleneck at ~789us/core in TimelineSim):
  - All dense x-pipeline matmuls (W_in, W_root, w_gate) run as exact fp16
    hi/lo 3-term products (Whi@xhi + Whi@xlo + Wlo@xhi, ~2^-22 rel error)
    instead of fp32 (3 cyc/col vs 4). x is carried as an fp16 hi/lo pair
    end-to-end; the pair is also what the gather table already needed.
  - MoE expert matmuls in plain fp16 (post-gate path, 5e-4 rel is fine).
  - cnt_inv broadcast via two fp16 rank-1 matmuls (hi+lo, exact); MoE gate
    broadcast via fp16 (g is exactly 0/1); expert-bias column via a single
    hi/lo rank-1 of (be2[0]-be2[1]) plus a per-partition Act bias.
  - xT inter-layer staging in DRAM as fp16 hi/lo pairs (half the bytes).
  - Export transposes run on fp16 (1 cyc/row) with one fused stage copy.
  - ps_sel pool bufs 2->4 so the two relations' selector accumulations
    don't serialize on PSUM banks.

Self-contained: hardcodes shapes; imports only installed packages.
"""

import numpy as np

N = 50000
E = 400000
D = 256
R = 2
NE = 2
OUT = 256
NCORES = 8
NLOC = N // NCORES  # 6250
WIN = 512
NWIN = (NLOC + WIN - 1) // WIN  # 13
CHUNK = 128
N_GRID = 128

# config
AG_CHUNKS = 2         # table chunk tensors / collectives per layer (>=2)
AG_OVERLAP = True     # emit chunk AllGathers inline with producing windows
N_SWDGE_Q = 2         # swdge queues for gathers
REPS = 1              # repeat whole body (marginal-cost timing)
SKIP_COLL = False     # timing-ablation only: skip collectives
SKIP_GATHER = False   # timing-ablation only: skip dma_gathers too

SELU_SCALE = 1.0507009873554805
SELU_ALPHA = 1.6732632423543772
NEG_SLOPE = 0.01

TW = 2 * D  # table row width in fp16 elements (hi 256 | lo 256)


def _chunk_bounds(agc):
    # WIN-aligned, slightly unbalanced splits minimize per-group ceil padding
    table = {
        2: [0, 3584, NLOC],
        3: [0, 2048, 4096, NLOC],
        4: [0, 1536, 3072, 4608, NLOC],
    }
    if agc in table:
        return table[agc]
    base = max(WIN, (NLOC // agc) // WIN * WIN)
    return [min(k * base, NLOC) for k in range(agc)] + [NLOC]


def _wrap_idx(idx):
    """int16 index list (len multiple of 16) -> (128, len/16) wrapped+replicated."""
    n = len(idx)
    w = idx.reshape(n // 16, 16).T.astype(np.int16)
    return np.tile(w, (8, 1))


# ----------------------------------------------------------------------------
# host-side planning
# ----------------------------------------------------------------------------

def build_plan(edge_index, edge_type):
    agc = AG_CHUNKS
    bounds = _chunk_bounds(agc)
    csz = [bounds[k + 1] - bounds[k] for k in range(agc)]

    src = np.asarray(edge_index[0], dtype=np.int64)
    dst = np.asarray(edge_index[1], dtype=np.int64)
    rel = np.asarray(edge_type, dtype=np.int64)

    core = dst // NLOC
    seg = rel * NLOC + (dst % NLOC)
    NSEG = R * NLOC

    cnt = np.bincount((core * NSEG + seg).astype(np.int64),
                      minlength=NCORES * NSEG).reshape(NCORES, NSEG)

    # chunk + row-in-chunk-tensor of each edge's source node
    src_c = src // NLOC
    src_j = src % NLOC
    src_k = np.searchsorted(bounds, src_j, side="right") - 1
    csz_arr = np.asarray(csz)
    b_arr = np.asarray(bounds[:-1])
    src_row = src_c * csz_arr[src_k] + (src_j - b_arr[src_k])

    # groups: (r, wbase, gbase, gsize, win_index)
    groups = []
    wi = 0
    for r in range(R):
        for w in range(NWIN):
            wb = w * WIN
            nw = min(WIN, NLOC - wb)
            g0 = 0
            while g0 < nw:
                gs = min(N_GRID, nw - g0)
                groups.append((r, wb, wb + g0, gs, wi))
                g0 += gs
            wi += 1
    n_windows_total = wi
    NG = len(groups)

    win_groups = [[] for _ in range(n_windows_total)]
    for gi, g in enumerate(groups):
        win_groups[g[4]].append(gi)

    # per-core edge lists sorted by (seg, chunk)
    per_core = []
    for c in range(NCORES):
        m = core == c
        s_seg, s_row, s_k = seg[m], src_row[m], src_k[m]
        o = np.lexsort((s_k, s_seg))
        per_core.append((s_seg[o], s_row[o], s_k[o]))

    # slots per (group, chunk): max over cores of ceil(count/128)
    n_gk = np.zeros((NG, agc), np.int64)
    core_group_edges = []  # [core][gi][k] -> (rows, seg_local)
    for c in range(NCORES):
        s_seg, s_row, s_k = per_core[c]
        lst = []
        for gi, (r, wb, gb, gs, _) in enumerate(groups):
            lo_b = np.searchsorted(s_seg, r * NLOC + gb)
            hi_b = np.searchsorted(s_seg, r * NLOC + gb + gs)
            rows, gg, kk = (s_row[lo_b:hi_b], s_seg[lo_b:hi_b] - (r * NLOC + gb),
                            s_k[lo_b:hi_b])
            per_k = []
            for k in range(agc):
                mk = kk == k
                per_k.append((rows[mk], gg[mk]))
                n_gk[gi, k] = max(n_gk[gi, k], -(-int(mk.sum()) // CHUNK))
            lst.append(per_k)
        core_group_edges.append(lst)

    # every (win, rel) needs >=1 slot so the PSUM bank gets cleared
    for w in range(n_windows_total):
        if sum(int(n_gk[gi, k]) for gi in win_groups[w] for k in range(agc)) == 0:
            n_gk[win_groups[w][0], 0] = 1

    # slot layout per (win, rel): chunk-major, then group order
    slot_group = []
    win_slot_chunks = []  # per win: list over k of slot count
    win_slot_start = []
    for w in range(n_windows_total):
        start = len(slot_group)
        per_k_counts = []
        for k in range(agc):
            nk = 0
            for gi in win_groups[w]:
                slot_group.extend([gi] * int(n_gk[gi, k]))
                nk += int(n_gk[gi, k])
            per_k_counts.append(nk)
        win_slot_start.append(start)
        win_slot_chunks.append(per_k_counts)
    n_slots = len(slot_group)

    idx_all = np.zeros((NCORES, n_slots, CHUNK), np.int16)
    seg_all = np.full((NCORES, CHUNK, n_slots), -1.0, np.float32)
    for c in range(NCORES):
        for w in range(n_windows_total):
            cursor = win_slot_start[w]
            for k in range(agc):
                for gi in win_groups[w]:
                    nsl = int(n_gk[gi, k])
                    if nsl == 0:
                        continue
                    rows, gg = core_group_edges[c][gi][k]
                    ne = len(rows)
                    pad = nsl * CHUNK - ne
                    rr = np.concatenate([rows, np.zeros(pad, np.int64)])
                    gp = np.concatenate([gg, np.full(pad, -1, np.int64)])
                    for s in range(nsl):
                        sl = cursor + s
                        idx_all[c, sl] = rr[s * CHUNK:(s + 1) * CHUNK].astype(np.int16)
                        seg_all[c, :, sl] = gp[s * CHUNK:(s + 1) * CHUNK].astype(np.float32)
                    cursor += nsl

    # idx16 column layout: per (win, rel), per chunk
    win_idx_cols = []  # per win: [(col, ni), ...] per chunk
    col = 0
    for w in range(n_windows_total):
        entry = []
        for k in range(agc):
            ni = win_slot_chunks[w][k] * CHUNK
            entry.append((col, ni))
            col += ni // 16
        win_idx_cols.append(entry)
    tot_cols = col

    idx16 = []
    for c in range(NCORES):
        buf = np.zeros((128, tot_cols), np.int16)
        for w in range(n_windows_total):
            cursor = win_slot_start[w]
            for k in range(agc):
                nsl = win_slot_chunks[w][k]
                ck, ni = win_idx_cols[w][k]
                if ni:
                    buf[:, ck:ck + ni // 16] = _wrap_idx(
                        idx_all[c, cursor:cursor + nsl].reshape(-1))
                cursor += nsl
        idx16.append(buf)

    # cnt_inv rows, fp16 hi + fp16 residual (exact through psum accumulate)
    cntinv = np.ones((NCORES, n_windows_total, WIN), np.float64)
    for c in range(NCORES):
        for r in range(R):
            for w in range(NWIN):
                wb = w * WIN
                nw = min(WIN, NLOC - wb)
                cc = cnt[c, r * NLOC + wb: r * NLOC + wb + nw]
                cntinv[c, r * NWIN + w, :nw] = 1.0 / np.maximum(cc, 1)
    ci32 = cntinv.astype(np.float32)
    ci_hi = ci32.astype(np.float16)
    ci_lo = (ci32 - ci_hi.astype(np.float32)).astype(np.float16)
    cntinv16 = np.stack([ci_hi, ci_lo], axis=2)  # (NCORES, NSEGW, 2, WIN)

    plan = dict(
        agc=agc, bounds=bounds, csz=csz,
        groups=groups,
        slot_group=np.array(slot_group, np.int64),
        win_groups=win_groups,
        win_slot_start=win_slot_start,
        win_slot_chunks=win_slot_chunks,
        win_idx_cols=win_idx_cols,
        n_slots=n_slots,
        idx_cols=tot_cols,
        n_windows_total=n_windows_total,
        max_slots=max(sum(cc) for cc in win_slot_chunks),
    )
    data = dict(idx16=idx16, seg_all=seg_all, cntinv16=cntinv16)
    return plan, data


# ----------------------------------------------------------------------------
# bass program
# ----------------------------------------------------------------------------

def build_nc(plan):
    import concourse.mybir as mybir
    import concourse.tile as tile
    from concourse import bacc
    from concourse.masks import make_identity

    dt = mybir.dt
    f32 = dt.float32
    f16 = dt.float16
    Alu = mybir.AluOpType
    ACT = mybir.ActivationFunctionType

    agc = plan["agc"]
    bounds = plan["bounds"]
    csz = plan["csz"]
    NSEGW = plan["n_windows_total"]
    NSLOT = plan["n_slots"]
    IDXC = plan["idx_cols"]
    MAX_SLOTS = plan["max_slots"]
    groups = plan["groups"]
    slot_group = plan["slot_group"]

    nc = bacc.Bacc(None, num_devices=NCORES, num_swdge_queues=N_SWDGE_Q)

    xcat_h_in = nc.dram_tensor("xcath", [128, 2, NLOC], f16, kind="ExternalInput")
    xcat_l_in = nc.dram_tensor("xcatl", [128, 2, NLOC], f16, kind="ExternalInput")
    idx16_in = nc.dram_tensor("idx16", [128, IDXC], dt.int16, kind="ExternalInput")
    seg_in = nc.dram_tensor("segloc", [128, NSLOT], f32, kind="ExternalInput")
    cntinv_in = nc.dram_tensor("cntinv16", [NSEGW, 2, WIN], f16,
                               kind="ExternalInput")
    w_in_in = nc.dram_tensor("w_in2", [2, 128, 2, D], f16, kind="ExternalInput")
    w_root_in = nc.dram_tensor("w_root2", [2, 128, 2, D], f16,
                               kind="ExternalInput")
    w_rel_in = nc.dram_tensor("w_rel", [R, 128, 2, D], f32, kind="ExternalInput")
    b_in_in = nc.dram_tensor("b_in", [128, 2], f32, kind="ExternalInput")
    b_rgcn_in = nc.dram_tensor("b_rgcn", [128, 2], f32, kind="ExternalInput")
    wg_in = nc.dram_tensor("wgate2", [2, 128, 2, 1], f16, kind="ExternalInput")
    we1_in = nc.dram_tensor("we1", [NE, 128, 2, D], f16, kind="ExternalInput")
    be1_in = nc.dram_tensor("be1", [NE, 128, 2], f32, kind="ExternalInput")
    we2_in = nc.dram_tensor("we2", [NE, 128, 2, OUT], f16, kind="ExternalInput")
    be2d_in = nc.dram_tensor("be2diff", [1, 2, 2, 128], f16,
                             kind="ExternalInput")  # (hi/lo, mc, col)
    be2b_in = nc.dram_tensor("be2base", [128, 2], f32, kind="ExternalInput")
    out_t = nc.dram_tensor("out", [OUT, NLOC], f32, kind="ExternalOutput")

    with tile.TileContext(nc) as tc:
        with (
            tc.tile_pool(name="const", bufs=1) as cpool,
            tc.tile_pool(name="work", bufs=2) as wpool,
            tc.tile_pool(name="slabp", bufs=3) as slabpool,
            tc.tile_pool(name="selp", bufs=4) as selpool,
            tc.tile_pool(name="stage", bufs=3) as stpool,
            tc.tile_pool(name="psum_sel", bufs=2, space="PSUM") as ps_sel,
            tc.tile_pool(name="psum_xf", bufs=2, space="PSUM") as ps_xf,
            tc.tile_pool(name="psum_misc", bufs=1, space="PSUM") as ps_misc,
            tc.tile_pool(name="dram", bufs=1, space="DRAM") as dpool,
            tc.tile_pool(name="dramsh", bufs=1, space="DRAM") as shpool,
        ):
            # constants / weights
            ident = cpool.tile([128, 128], f32)
            make_identity(nc, ident[:])
            ident16 = cpool.tile([128, 128], f16)
            nc.vector.tensor_copy(ident16[:], ident[:])
            iota_i = cpool.tile([128, N_GRID], dt.int32)
            nc.gpsimd.iota(iota_i[:], pattern=[[1, N_GRID]], base=0,
                           channel_multiplier=0)
            iota_f = cpool.tile([128, N_GRID], f32)
            nc.vector.tensor_copy(iota_f[:], iota_i[:])
            ones16 = cpool.tile([1, 128], f16)
            nc.vector.memset(ones16[:], 1.0)

            def load_const(t_in, shape, re=None, tag=None, cdt=f32):
                t = cpool.tile(shape, cdt, tag=tag)
                nc.sync.dma_start(t[:], t_in[:] if re is None else t_in[:].rearrange(re))
                return t

            # dense weights: [hilo, 128, 2, D] -> sbuf [128, hilo, 2, D]
            w_in_sb = load_const(w_in_in, [128, 2, 2, D], "h p k d -> p h k d",
                                 tag="w_in", cdt=f16)
            w_root_sb = load_const(w_root_in, [128, 2, 2, D],
                                   "h p k d -> p h k d", tag="w_root", cdt=f16)
            w_rel_sb = load_const(w_rel_in, [128, R, 2, D], "r p k d -> p r k d",
                                  tag="w_rel")
            b_in_sb = load_const(b_in_in, [128, 2], tag="b_in")
            b_rg_sb = load_const(b_rgcn_in, [128, 2], tag="b_rg")
            wgd_sb = load_const(wg_in, [128, 2, 2, 1], "h p k d -> p h k d",
                                tag="wgd", cdt=f16)
            we1_sb = load_const(we1_in, [128, NE, 2, D], "e p k d -> p e k d",
                                tag="we1", cdt=f16)
            be1_sb = load_const(be1_in, [128, NE, 2], "e p k -> p e k", tag="be1")
            we2_sb = load_const(we2_in, [128, NE, 2, OUT], "e p k d -> p e k d",
                                tag="we2", cdt=f16)
            be2d_sb = cpool.tile([1, 2, 2, 128], f16)
            nc.sync.dma_start(be2d_sb[:], be2d_in[:])
            be2b_sb = load_const(be2b_in, [128, 2], tag="be2b")

            seg_sb = cpool.tile([128, NSLOT], f32)
            nc.sync.dma_start(seg_sb[:], seg_in[:])
            idx_sb = cpool.tile([128, IDXC], dt.int16)
            nc.sync.dma_start(idx_sb[:], idx16_in[:])

            # DRAM staging: fp16 hi/lo pairs for inter-layer x
            xT1h = dpool.tile([128, 2, NLOC], f16)
            xT1l = dpool.tile([128, 2, NLOC], f16)
            xT2h = dpool.tile([128, 2, NLOC], f16)
            xT2l = dpool.tile([128, 2, NLOC], f16)
            xloc1 = dpool.tile([NLOC, TW], f16)
            xloc2 = dpool.tile([NLOC, TW], f16)
            xfulls = []  # [rep][layer][chunk]
            for _r in range(REPS):
                per_layer = []
                for li in (1, 2):
                    per_layer.append([
                        shpool.tile([NCORES * csz[k], TW], f16,
                                    addr_space="Shared",
                                    name="xf%d_%d_%d" % (li, _r, k))
                        for k in range(agc)
                    ])
                xfulls.append(per_layer)

            def win_sizes(w):
                wb = w * WIN
                return wb, min(WIN, NLOC - wb)

            def mm3(ps, w_sb, xh, xlo, mc, nw, start, stop):
                """3-term hi/lo dense matmul: out[mc] += W.T @ x (exact).

                w_sb: [128, hilo, 2, D] fp16; xh/xlo: [128, 2, nw] fp16.
                """
                for kc in range(2):
                    first = start and kc == 0
                    nc.tensor.matmul(
                        ps, w_sb[:, 0, kc, mc * 128:(mc + 1) * 128],
                        xh[:, kc, :nw], start=first, stop=False)
                    nc.tensor.matmul(
                        ps, w_sb[:, 0, kc, mc * 128:(mc + 1) * 128],
                        xlo[:, kc, :nw], start=False, stop=False)
                    last = stop and kc == 1
                    nc.tensor.matmul(
                        ps, w_sb[:, 1, kc, mc * 128:(mc + 1) * 128],
                        xh[:, kc, :nw], start=False, stop=last)

            def export_window(xh, xlo, wb, nw, xloc):
                # fp16 transpose (128, 2, nw) hi+lo -> node-major rows of xloc.
                # One packed PSUM tile = 1 bank; the two halves double-buffer
                # the per-128-node blocks manually.
                pst2 = ps_misc.tile([128, 2, TW], f16, space="PSUM",
                                    tag="misc16")
                nb = 0
                blk = 0
                while nb < nw:
                    bs = min(128, nw - nb)
                    stg = stpool.tile([128, TW], f16, tag="stage")
                    pst = pst2[:, blk % 2, :]
                    for mc in range(2):
                        nc.tensor.transpose(pst[:bs, mc * 128:(mc + 1) * 128],
                                            xh[:, mc, nb:nb + bs], ident16[:])
                        nc.tensor.transpose(pst[:bs, D + mc * 128:D + (mc + 1) * 128],
                                            xlo[:, mc, nb:nb + bs], ident16[:])
                    nc.scalar.activation(stg[:bs, :], pst[:bs, :], ACT.Copy)
                    nc.sync.dma_start(xloc[wb + nb: wb + nb + bs, :], stg[:bs, :])
                    nb += bs
                    blk += 1

            for _rep in range(REPS):
                xfull1, xfull2 = xfulls[_rep]
                ag_state1, ag_state2 = [0], [0]

                def ag_emit(xloc, xflist, rows_done, state):
                    while state[0] < agc and bounds[state[0] + 1] <= rows_done:
                        k = state[0]
                        if not SKIP_COLL:
                            nc.gpsimd.collective_compute(
                                "AllGather", mybir.AluOpType.bypass,
                                replica_groups=[list(range(NCORES))],
                                ins=[xloc[bounds[k]:bounds[k + 1], :].opt()],
                                outs=[xflist[k][:].opt()])
                        state[0] += 1

                # ------------ layer 0: x1 = selu(x_cat @ W_in + b_in) --------
                for w in range(NWIN):
                    wb, nw = win_sizes(w)
                    xwh = wpool.tile([128, 2, WIN], f16, tag="xwh")
                    xwl = wpool.tile([128, 2, WIN], f16, tag="xwl")
                    nc.sync.dma_start(xwh[:, :, :nw], xcat_h_in[:, :, wb:wb + nw])
                    nc.sync.dma_start(xwl[:, :, :nw], xcat_l_in[:, :, wb:wb + nw])
                    xoh = wpool.tile([128, 2, WIN], f16, tag="xoh")
                    xol = wpool.tile([128, 2, WIN], f16, tag="xol")
                    for mc in range(2):
                        ps = ps_xf.tile([128, WIN], f32, space="PSUM", tag="xf")
                        mm3(ps[:, :nw], w_in_sb, xwh, xwl, mc, nw,
                            start=True, stop=True)
                        pos = wpool.tile([128, WIN], f32, tag="selu_pos")
                        nc.vector.tensor_scalar(
                            out=pos[:, :nw], in0=ps[:, :nw],
                            scalar1=b_in_sb[:, mc:mc + 1], scalar2=0.0,
                            op0=Alu.add, op1=Alu.max)
                        neg = wpool.tile([128, WIN], f32, tag="selu_neg")
                        nc.vector.tensor_scalar(
                            out=neg[:, :nw], in0=ps[:, :nw],
                            scalar1=b_in_sb[:, mc:mc + 1], scalar2=0.0,
                            op0=Alu.add, op1=Alu.min)
                        e = wpool.tile([128, WIN], f32, tag="selu_e")
                        nc.scalar.activation(e[:, :nw], neg[:, :nw], ACT.Exp)
                        sa = SELU_SCALE * SELU_ALPHA
                        nc.vector.tensor_scalar(
                            out=e[:, :nw], in0=e[:, :nw], scalar1=sa, scalar2=sa,
                            op0=Alu.mult, op1=Alu.subtract)
                        nc.vector.tensor_scalar(
                            out=pos[:, :nw], in0=pos[:, :nw],
                            scalar1=SELU_SCALE, scalar2=None, op0=Alu.mult)
                        xo = wpool.tile([128, WIN], f32, tag="xo32")
                        nc.vector.tensor_tensor(
                            out=xo[:, :nw], in0=pos[:, :nw], in1=e[:, :nw],
                            op=Alu.add)
                        nc.scalar.activation(xoh[:, mc, :nw], xo[:, :nw],
                                             ACT.Copy)
                        nc.vector.tensor_tensor(
                            out=xol[:, mc, :nw], in0=xo[:, :nw],
                            in1=xoh[:, mc, :nw], op=Alu.subtract)
                    nc.sync.dma_start(xT1h[:, :, wb:wb + nw], xoh[:, :, :nw])
                    nc.sync.dma_start(xT1l[:, :, wb:wb + nw], xol[:, :, :nw])
                    export_window(xoh, xol, wb, nw, xloc1)
                    if AG_OVERLAP:
                        ag_emit(xloc1, xfull1, wb + nw, ag_state1)
                ag_emit(xloc1, xfull1, NLOC, ag_state1)

                # ------------ rgcn layers ------------
                def rgcn_layer(xfull, xTh_cur, xTl_cur, xTh_next, xTl_next,
                               xloc_next, li, moe_fn=None, xfull_next=None,
                               ag_state=None):
                    for w in range(NWIN):
                        wb, nw = win_sizes(w)
                        s_tiles = {}
                        for r in range(R):
                            wi = r * NWIN + w
                            cursor = plan["win_slot_start"][wi]
                            kcounts = plan["win_slot_chunks"][wi]
                            nslots_w = sum(kcounts)
                            slab = slabpool.tile([128, MAX_SLOTS, TW], f16,
                                                 tag="slab")
                            off = 0
                            for k in range(agc):
                                ck, ni = plan["win_idx_cols"][wi][k]
                                nsl = kcounts[k]
                                if ni and not SKIP_GATHER:
                                    nc.gpsimd.dma_gather(
                                        out_ap=slab[:, off:off + nsl, :],
                                        in_ap=xfull[k][:],
                                        idxs_ap=idx_sb[:, ck:ck + ni // 16],
                                        num_idxs=ni, num_idxs_reg=ni,
                                        elem_size=TW, single_packet=False,
                                        queue_num=(r * agc + k) % N_SWDGE_Q)
                                off += nsl
                            ps0 = ps_sel.tile([128, WIN], f32, space="PSUM",
                                              tag="sel0")
                            ps1 = ps_sel.tile([128, WIN], f32, space="PSUM",
                                              tag="sel1")
                            for s in range(nslots_w):
                                sl = cursor + s
                                gi = int(slot_group[sl])
                                gb_in_win = groups[gi][2] - wb
                                gs = groups[gi][3]
                                sel = selpool.tile([128, N_GRID], f16, tag="sel")
                                nc.vector.tensor_scalar(
                                    out=sel[:, :gs], in0=iota_f[:, :gs],
                                    scalar1=seg_sb[:, sl:sl + 1], scalar2=None,
                                    op0=Alu.is_equal)
                                cols = slice(gb_in_win, gb_in_win + gs)
                                last = s == nslots_w - 1
                                nc.tensor.matmul(
                                    ps0[:, cols], slab[:, s, 0:128],
                                    sel[:, :gs], start=(s == 0), stop=False)
                                nc.tensor.matmul(
                                    ps0[:, cols], slab[:, s, 256:384],
                                    sel[:, :gs], start=False, stop=last)
                                nc.tensor.matmul(
                                    ps1[:, cols], slab[:, s, 128:256],
                                    sel[:, :gs], start=(s == 0), stop=False)
                                nc.tensor.matmul(
                                    ps1[:, cols], slab[:, s, 384:512],
                                    sel[:, :gs], start=False, stop=last)
                            # broadcast cnt_inv to 128 rows: two fp16 rank-1
                            # matmuls (hi+lo) accumulate the exact f32 value
                            ci16 = wpool.tile([1, 2, WIN], f16, tag="ci_row")
                            nc.sync.dma_start(ci16[:], cntinv_in[wi:wi + 1])
                            cib = ps_misc.tile([128, WIN], f32, space="PSUM",
                                               tag="misc")
                            nc.tensor.matmul(cib[:, :nw], ones16[:],
                                             ci16[0:1, 0, :nw], start=True,
                                             stop=False)
                            nc.tensor.matmul(cib[:, :nw], ones16[:],
                                             ci16[0:1, 1, :nw], start=False,
                                             stop=True)
                            ci = wpool.tile([128, WIN], f32, tag="cntinv")
                            nc.scalar.activation(ci[:, :nw], cib[:, :nw], ACT.Copy)
                            s0 = wpool.tile([128, WIN], f32, tag="s0_%d" % r)
                            s1 = wpool.tile([128, WIN], f32, tag="s1_%d" % r)
                            nc.vector.tensor_tensor(out=s0[:, :nw], in0=ps0[:, :nw],
                                                    in1=ci[:, :nw], op=Alu.mult)
                            nc.vector.tensor_tensor(out=s1[:, :nw], in0=ps1[:, :nw],
                                                    in1=ci[:, :nw], op=Alu.mult)
                            s_tiles[r] = (s0, s1)

                        xwh = wpool.tile([128, 2, WIN], f16, tag="xwh")
                        xwl = wpool.tile([128, 2, WIN], f16, tag="xwl")
                        nc.sync.dma_start(xwh[:, :, :nw],
                                          xTh_cur[:, :, wb:wb + nw])
                        nc.sync.dma_start(xwl[:, :, :nw],
                                          xTl_cur[:, :, wb:wb + nw])
                        xoh = wpool.tile([128, 2, WIN], f16, tag="xoh")
                        xol = wpool.tile([128, 2, WIN], f16, tag="xol")
                        for mc in range(2):
                            ps = ps_xf.tile([128, WIN], f32, space="PSUM", tag="xf")
                            mm3(ps[:, :nw], w_root_sb, xwh, xwl, mc, nw,
                                start=True, stop=False)
                            for r in range(R):
                                for kc in range(2):
                                    st = s_tiles[r][kc]
                                    nc.tensor.matmul(
                                        ps[:, :nw],
                                        w_rel_sb[:, r, kc, mc * 128:(mc + 1) * 128],
                                        st[:, :nw],
                                        start=False, stop=(r == R - 1 and kc == 1))
                            # x = ps + b, split to fp16 hi/lo (Identity
                            # accepts an AP bias; Copy does not)
                            nc.scalar.activation(xoh[:, mc, :nw], ps[:, :nw],
                                                 ACT.Identity,
                                                 bias=b_rg_sb[:, mc:mc + 1])
                            xf = wpool.tile([128, WIN], f32, tag="xf32")
                            nc.vector.tensor_scalar(
                                out=xf[:, :nw], in0=ps[:, :nw],
                                scalar1=b_rg_sb[:, mc:mc + 1], scalar2=None,
                                op0=Alu.add)
                            nc.vector.tensor_tensor(
                                out=xol[:, mc, :nw], in0=xf[:, :nw],
                                in1=xoh[:, mc, :nw], op=Alu.subtract)
                        if xTh_next is not None:
                            nc.sync.dma_start(xTh_next[:, :, wb:wb + nw],
                                              xoh[:, :, :nw])
                            nc.sync.dma_start(xTl_next[:, :, wb:wb + nw],
                                              xol[:, :, :nw])
                        if xloc_next is not None:
                            export_window(xoh, xol, wb, nw, xloc_next)
                            if xfull_next is not None and AG_OVERLAP:
                                ag_emit(xloc_next, xfull_next, wb + nw, ag_state)
                        if moe_fn is not None:
                            moe_fn(xoh, xol, wb, nw)
                    if xfull_next is not None:
                        ag_emit(xloc_next, xfull_next, NLOC, ag_state)

                # ------------ MoE (fused into layer-2 windows) ------------
                def moe_window(xh, xlo, wb, nw):
                    psl = ps_misc.tile([128, WIN], f32, space="PSUM", tag="misc")
                    for kc in range(2):
                        nc.tensor.matmul(psl[:1, :nw], wgd_sb[:, 0, kc, :],
                                         xh[:, kc, :nw], start=(kc == 0),
                                         stop=False)
                        nc.tensor.matmul(psl[:1, :nw], wgd_sb[:, 0, kc, :],
                                         xlo[:, kc, :nw], start=False, stop=False)
                        nc.tensor.matmul(psl[:1, :nw], wgd_sb[:, 1, kc, :],
                                         xh[:, kc, :nw], start=False,
                                         stop=(kc == 1))
                    g_row = wpool.tile([1, WIN], f32, tag="grow")
                    nc.vector.tensor_scalar(out=g_row[:, :nw], in0=psl[:1, :nw],
                                            scalar1=0.0, scalar2=None,
                                            op0=Alu.is_ge)
                    g16 = wpool.tile([1, WIN], f16, tag="g16")
                    nc.scalar.activation(g16[:, :nw], g_row[:, :nw], ACT.Copy)
                    psb = ps_misc.tile([128, WIN], f32, space="PSUM", tag="misc")
                    nc.tensor.matmul(psb[:, :nw], ones16[:], g16[:, :nw],
                                     start=True, stop=True)
                    gb = wpool.tile([128, WIN], f32, tag="gb_sb")
                    nc.scalar.activation(gb[:, :nw], psb[:, :nw], ACT.Copy)
                    ginv = wpool.tile([128, WIN], f32, tag="ginv")
                    nc.vector.tensor_scalar(out=ginv[:, :nw], in0=gb[:, :nw],
                                            scalar1=-1.0, scalar2=1.0,
                                            op0=Alu.mult, op1=Alu.add)

                    h1g = {}
                    for e in range(NE):
                        for mc in range(2):
                            psh = ps_xf.tile([128, WIN], f32, space="PSUM",
                                             tag="xf")
                            for kc in range(2):
                                nc.tensor.matmul(
                                    psh[:, :nw],
                                    we1_sb[:, e, kc, mc * 128:(mc + 1) * 128],
                                    xh[:, kc, :nw],
                                    start=(kc == 0), stop=(kc == 1))
                            h = wpool.tile([128, WIN], f16,
                                           tag="h1_%d_%d" % (e, mc))
                            nc.scalar.activation(
                                h[:, :nw], psh[:, :nw], ACT.Lrelu,
                                bias=be1_sb[:, e, mc:mc + 1], alpha=NEG_SLOPE)
                            gt = gb if e == 0 else ginv
                            nc.vector.tensor_tensor(out=h[:, :nw], in0=h[:, :nw],
                                                    in1=gt[:, :nw], op=Alu.mult)
                            h1g[(e, mc)] = h
                    for mc in range(2):
                        psy = ps_xf.tile([128, WIN], f32, space="PSUM", tag="xf")
                        first = True
                        for e in range(NE):
                            for kc in range(2):
                                nc.tensor.matmul(
                                    psy[:, :nw],
                                    we2_sb[:, e, kc, mc * 128:(mc + 1) * 128],
                                    h1g[(e, kc)][:, :nw],
                                    start=first, stop=False)
                                first = False
                        # + (be2[0]-be2[1]) (x) g  via hi/lo rank-1 fp16
                        nc.tensor.matmul(psy[:, :nw], be2d_sb[0:1, 0, mc, :],
                                         g16[:, :nw], start=False, stop=False)
                        nc.tensor.matmul(psy[:, :nw], be2d_sb[0:1, 1, mc, :],
                                         g16[:, :nw], start=False, stop=True)
                        yt = wpool.tile([128, WIN], f32, tag="yt")
                        nc.scalar.activation(yt[:, :nw], psy[:, :nw],
                                             ACT.Identity,
                                             bias=be2b_sb[:, mc:mc + 1])
                        nc.sync.dma_start(out_t[mc * 128:(mc + 1) * 128,
                                                wb:wb + nw],
                                          yt[:, :nw])

                rgcn_layer(xfull1, xT1h, xT1l, xT2h, xT2l, xloc2, 1,
                           xfull_next=xfull2, ag_state=ag_state2)
                rgcn_layer(xfull2, xT2h, xT2l, None, None, None, 2,
                           moe_fn=moe_window)

    nc.compile()
    return nc


# ----------------------------------------------------------------------------
# entry point
# ----------------------------------------------------------------------------

def _hilo(a):
    a = np.asarray(a, np.float32)
    hi = a.astype(np.float16)
    lo = (a - hi.astype(np.float32)).astype(np.float16)
    return hi, lo


def kernel(des, tweet, num_prop, cat_prop, edge_index, edge_type,
           W_in, b_in, W_rel, W_root, b_rgcn, w_gate, We1, be1, We2, be2):
    from concourse.bass_utils import run_bass_kernel_spmd

    x_cat = np.concatenate(
        [np.asarray(des), np.asarray(tweet), np.asarray(num_prop),
         np.asarray(cat_prop)], axis=1).astype(np.float32)

    plan, data = build_plan(np.asarray(edge_index), np.asarray(edge_type))
    nc = build_nc(plan)

    def wmat(w):
        w = np.asarray(w, np.float32)
        return np.ascontiguousarray(w.reshape(2, 128, w.shape[1]).transpose(1, 0, 2))

    def wmat2(w):
        return np.stack(_hilo(wmat(w)))  # (2=hi/lo, 128, 2, D)

    def bvec(b):
        return np.ascontiguousarray(np.asarray(b, np.float32).reshape(2, 128).T)

    w_rel_h = np.stack([wmat(np.asarray(W_rel)[r]) for r in range(R)])
    we1_h = np.stack([wmat(np.asarray(We1)[e]).astype(np.float16)
                      for e in range(NE)])
    be1_h = np.stack([bvec(np.asarray(be1)[e]) for e in range(NE)])
    we2_h = np.stack([wmat(np.asarray(We2)[e]).astype(np.float16)
                      for e in range(NE)])
    be2 = np.asarray(be2, np.float32)
    be2d_hi, be2d_lo = _hilo((be2[0] - be2[1]).reshape(1, 2, 128))
    be2d = np.stack([be2d_hi, be2d_lo], axis=1)  # (1, 2hilo, 2mc, 128)
    be2b = bvec(be2[1])

    wg = np.asarray(w_gate, np.float32)
    wgd2 = np.stack(_hilo(wmat(wg[:, 0:1] - wg[:, 1:2])))

    in_maps = []
    for c in range(NCORES):
        xc = x_cat[c * NLOC:(c + 1) * NLOC]
        xcatT = np.ascontiguousarray(xc.T.reshape(2, 128, NLOC).transpose(1, 0, 2))
        xch, xcl = _hilo(xcatT)
        in_maps.append({
            "xcath": xch,
            "xcatl": xcl,
            "idx16": data["idx16"][c],
            "segloc": data["seg_all"][c],
            "cntinv16": data["cntinv16"][c],
            "w_in2": wmat2(W_in), "w_root2": wmat2(W_root), "w_rel": w_rel_h,
            "b_in": bvec(b_in), "b_rgcn": bvec(b_rgcn),
            "wgate2": wgd2,
            "we1": we1_h, "be1": be1_h,
            "we2": we2_h, "be2diff": be2d, "be2base": be2b,
        })

    res = run_bass_kernel_spmd(nc, in_maps, core_ids=list(range(NCORES)))
    global last_nc, last_in_maps
    last_nc, last_in_maps = nc, in_maps
    y = np.concatenate([res.results[c]["out"].T for c in range(NCORES)], axis=0)
    return y.astype(np.float32)


last_nc = None
last_in_maps = None


# revision 20
# speedup vs baseline: 50.2669x; 1.0037x over previous
"""BotRGCN + MoE (top-1 of 2) Trainium2 Bass kernel, 8-core SPMD. v3.

v3 over v2 (PE was the bottleneck at ~789us/core in TimelineSim):
  - All dense x-pipeline matmuls (W_in, W_root, w_gate) run as exact fp16
    hi/lo 3-term products (Whi@xhi + Whi@xlo + Wlo@xhi, ~2^-22 rel error)
    instead of fp32 (3 cyc/col vs 4). x is carried as an fp16 hi/lo pair
    end-to-end; the pair is also what the gather table already needed.
  - MoE expert matmuls in plain fp16 (post-gate path, 5e-4 rel is fine).
  - cnt_inv broadcast via two fp16 rank-1 matmuls (hi+lo, exact); MoE gate
    broadcast via fp16 (g is exactly 0/1); expert-bias column via a single
    hi/lo rank-1 of (be2[0]-be2[1]) plus a per-partition Act bias.
  - xT inter-layer staging in DRAM as fp16 hi/lo pairs (half the bytes).
  - Export transposes run on fp16 (1 cyc/row) with one fused stage copy.
  - ps_sel pool bufs 2->4 so the two relations' selector accumulations
    don't serialize on PSUM banks.

Self-contained: hardcodes shapes; imports only installed packages.
"""

import numpy as np

N = 50000
E = 400000
D = 256
R = 2
NE = 2
OUT = 256
NCORES = 8
NLOC = N // NCORES  # 6250
WIN = 512
NWIN = (NLOC + WIN - 1) // WIN  # 13
CHUNK = 128
N_GRID = 128

# config
AG_CHUNKS = 2         # table chunk tensors / collectives per layer (>=2)
AG_OVERLAP = True     # emit chunk AllGathers inline with producing windows
N_SWDGE_Q = 4         # swdge queues for gathers
REPS = 1              # repeat whole body (marginal-cost timing)
SKIP_COLL = False     # timing-ablation only: skip collectives
SKIP_GATHER = False   # timing-ablation only: skip dma_gathers too
SLOT_CAP = None       # timing-ablation only: cap selector slots per (win,rel)

SELU_SCALE = 1.0507009873554805
SELU_ALPHA = 1.6732632423543772
NEG_SLOPE = 0.01

TW = 2 * D  # table row width in fp16 elements (hi 256 | lo 256)


def _chunk_bounds(agc):
    # WIN-aligned, slightly unbalanced splits minimize per-group ceil padding
    table = {
        2: [0, 3584, NLOC],
        3: [0, 2048, 4096, NLOC],
        4: [0, 1536, 3072, 4608, NLOC],
    }
    if agc in table:
        return table[agc]
    base = max(WIN, (NLOC // agc) // WIN * WIN)
    return [min(k * base, NLOC) for k in range(agc)] + [NLOC]


def _wrap_idx(idx):
    """int16 index list (len multiple of 16) -> (128, len/16) wrapped+replicated."""
    n = len(idx)
    w = idx.reshape(n // 16, 16).T.astype(np.int16)
    return np.tile(w, (8, 1))


# ----------------------------------------------------------------------------
# host-side planning
# ----------------------------------------------------------------------------

def build_plan(edge_index, edge_type):
    agc = AG_CHUNKS
    bounds = _chunk_bounds(agc)
    csz = [bounds[k + 1] - bounds[k] for k in range(agc)]

    src = np.asarray(edge_index[0], dtype=np.int64)
    dst = np.asarray(edge_index[1], dtype=np.int64)
    rel = np.asarray(edge_type, dtype=np.int64)

    core = dst // NLOC
    seg = rel * NLOC + (dst % NLOC)
    NSEG = R * NLOC

    cnt = np.bincount((core * NSEG + seg).astype(np.int64),
                      minlength=NCORES * NSEG).reshape(NCORES, NSEG)

    # chunk + row-in-chunk-tensor of each edge's source node
    src_c = src // NLOC
    src_j = src % NLOC
    src_k = np.searchsorted(bounds, src_j, side="right") - 1
    csz_arr = np.asarray(csz)
    b_arr = np.asarray(bounds[:-1])
    src_row = src_c * csz_arr[src_k] + (src_j - b_arr[src_k])

    # groups: (r, wbase, gbase, gsize, win_index)
    groups = []
    wi = 0
    for r in range(R):
        for w in range(NWIN):
            wb = w * WIN
            nw = min(WIN, NLOC - wb)
            g0 = 0
            while g0 < nw:
                gs = min(N_GRID, nw - g0)
                groups.append((r, wb, wb + g0, gs, wi))
                g0 += gs
            wi += 1
    n_windows_total = wi
    NG = len(groups)

    win_groups = [[] for _ in range(n_windows_total)]
    for gi, g in enumerate(groups):
        win_groups[g[4]].append(gi)

    # per-core edge lists sorted by (seg, chunk)
    per_core = []
    for c in range(NCORES):
        m = core == c
        s_seg, s_row, s_k = seg[m], src_row[m], src_k[m]
        o = np.lexsort((s_k, s_seg))
        per_core.append((s_seg[o], s_row[o], s_k[o]))

    # slots per (group, chunk): max over cores of ceil(count/128)
    n_gk = np.zeros((NG, agc), np.int64)
    core_group_edges = []  # [core][gi][k] -> (rows, seg_local)
    for c in range(NCORES):
        s_seg, s_row, s_k = per_core[c]
        lst = []
        for gi, (r, wb, gb, gs, _) in enumerate(groups):
            lo_b = np.searchsorted(s_seg, r * NLOC + gb)
            hi_b = np.searchsorted(s_seg, r * NLOC + gb + gs)
            rows, gg, kk = (s_row[lo_b:hi_b], s_seg[lo_b:hi_b] - (r * NLOC + gb),
                            s_k[lo_b:hi_b])
            per_k = []
            for k in range(agc):
                mk = kk == k
                per_k.append((rows[mk], gg[mk]))
                n_gk[gi, k] = max(n_gk[gi, k], -(-int(mk.sum()) // CHUNK))
            lst.append(per_k)
        core_group_edges.append(lst)

    # every (win, rel) needs >=1 slot so the PSUM bank gets cleared
    for w in range(n_windows_total):
        if sum(int(n_gk[gi, k]) for gi in win_groups[w] for k in range(agc)) == 0:
            n_gk[win_groups[w][0], 0] = 1

    # slot layout per (win, rel): chunk-major, then group order
    slot_group = []
    win_slot_chunks = []  # per win: list over k of slot count
    win_slot_start = []
    for w in range(n_windows_total):
        start = len(slot_group)
        per_k_counts = []
        for k in range(agc):
            nk = 0
            for gi in win_groups[w]:
                slot_group.extend([gi] * int(n_gk[gi, k]))
                nk += int(n_gk[gi, k])
            per_k_counts.append(nk)
        win_slot_start.append(start)
        win_slot_chunks.append(per_k_counts)
    n_slots = len(slot_group)

    idx_all = np.zeros((NCORES, n_slots, CHUNK), np.int16)
    seg_all = np.full((NCORES, CHUNK, n_slots), -1.0, np.float32)
    for c in range(NCORES):
        for w in range(n_windows_total):
            cursor = win_slot_start[w]
            for k in range(agc):
                for gi in win_groups[w]:
                    nsl = int(n_gk[gi, k])
                    if nsl == 0:
                        continue
                    rows, gg = core_group_edges[c][gi][k]
                    ne = len(rows)
                    pad = nsl * CHUNK - ne
                    rr = np.concatenate([rows, np.zeros(pad, np.int64)])
                    gp = np.concatenate([gg, np.full(pad, -1, np.int64)])
                    for s in range(nsl):
                        sl = cursor + s
                        idx_all[c, sl] = rr[s * CHUNK:(s + 1) * CHUNK].astype(np.int16)
                        seg_all[c, :, sl] = gp[s * CHUNK:(s + 1) * CHUNK].astype(np.float32)
                    cursor += nsl

    # idx16 column layout: per (win, rel), per chunk
    win_idx_cols = []  # per win: [(col, ni), ...] per chunk
    col = 0
    for w in range(n_windows_total):
        entry = []
        for k in range(agc):
            ni = win_slot_chunks[w][k] * CHUNK
            entry.append((col, ni))
            col += ni // 16
        win_idx_cols.append(entry)
    tot_cols = col

    idx16 = []
    for c in range(NCORES):
        buf = np.zeros((128, tot_cols), np.int16)
        for w in range(n_windows_total):
            cursor = win_slot_start[w]
            for k in range(agc):
                nsl = win_slot_chunks[w][k]
                ck, ni = win_idx_cols[w][k]
                if ni:
                    buf[:, ck:ck + ni // 16] = _wrap_idx(
                        idx_all[c, cursor:cursor + nsl].reshape(-1))
                cursor += nsl
        idx16.append(buf)

    # cnt_inv rows, fp16 hi + fp16 residual (exact through psum accumulate)
    cntinv = np.ones((NCORES, n_windows_total, WIN), np.float64)
    for c in range(NCORES):
        for r in range(R):
            for w in range(NWIN):
                wb = w * WIN
                nw = min(WIN, NLOC - wb)
                cc = cnt[c, r * NLOC + wb: r * NLOC + wb + nw]
                cntinv[c, r * NWIN + w, :nw] = 1.0 / np.maximum(cc, 1)
    ci32 = cntinv.astype(np.float32)
    ci_hi = ci32.astype(np.float16)
    ci_lo = (ci32 - ci_hi.astype(np.float32)).astype(np.float16)
    cntinv16 = np.stack([ci_hi, ci_lo], axis=2)  # (NCORES, NSEGW, 2, WIN)

    plan = dict(
        agc=agc, bounds=bounds, csz=csz,
        groups=groups,
        slot_group=np.array(slot_group, np.int64),
        win_groups=win_groups,
        win_slot_start=win_slot_start,
        win_slot_chunks=win_slot_chunks,
        win_idx_cols=win_idx_cols,
        n_slots=n_slots,
        idx_cols=tot_cols,
        n_windows_total=n_windows_total,
        max_slots=max(sum(cc) for cc in win_slot_chunks),
    )
    data = dict(idx16=idx16, seg_all=seg_all, cntinv16=cntinv16)
    return plan, data


# ----------------------------------------------------------------------------
# bass program
# ----------------------------------------------------------------------------

def build_nc(plan):
    import concourse.mybir as mybir
    import concourse.tile as tile
    from concourse import bacc
    from concourse.masks import make_identity

    dt = mybir.dt
    f32 = dt.float32
    f16 = dt.float16
    Alu = mybir.AluOpType
    ACT = mybir.ActivationFunctionType

    agc = plan["agc"]
    bounds = plan["bounds"]
    csz = plan["csz"]
    NSEGW = plan["n_windows_total"]
    NSLOT = plan["n_slots"]
    IDXC = plan["idx_cols"]
    MAX_SLOTS = plan["max_slots"]
    groups = plan["groups"]
    slot_group = plan["slot_group"]

    nc = bacc.Bacc(None, num_devices=NCORES, num_swdge_queues=N_SWDGE_Q)

    xcat_h_in = nc.dram_tensor("xcath", [128, 2, NLOC], f16, kind="ExternalInput")
    xcat_l_in = nc.dram_tensor("xcatl", [128, 2, NLOC], f16, kind="ExternalInput")
    idx16_in = nc.dram_tensor("idx16", [128, IDXC], dt.int16, kind="ExternalInput")
    seg_in = nc.dram_tensor("segloc", [128, NSLOT], f32, kind="ExternalInput")
    cntinv_in = nc.dram_tensor("cntinv16", [1, NSEGW, 2, WIN], f16,
                               kind="ExternalInput")
    w_in_in = nc.dram_tensor("w_in2", [2, 128, 2, D], f16, kind="ExternalInput")
    w_root_in = nc.dram_tensor("w_root2", [2, 128, 2, D], f16,
                               kind="ExternalInput")
    w_rel_in = nc.dram_tensor("w_rel", [R, 128, 2, D], f32, kind="ExternalInput")
    b_in_in = nc.dram_tensor("b_in", [128, 2], f32, kind="ExternalInput")
    b_rgcn_in = nc.dram_tensor("b_rgcn", [128, 2], f32, kind="ExternalInput")
    wg_in = nc.dram_tensor("wgate2", [2, 128, 2, 1], f16, kind="ExternalInput")
    we1_in = nc.dram_tensor("we1", [NE, 128, 2, D], f16, kind="ExternalInput")
    be1_in = nc.dram_tensor("be1", [NE, 128, 2], f32, kind="ExternalInput")
    we2_in = nc.dram_tensor("we2", [NE, 128, 2, OUT], f16, kind="ExternalInput")
    be2d_in = nc.dram_tensor("be2diff", [1, 2, 2, 128], f16,
                             kind="ExternalInput")  # (hi/lo, mc, col)
    be2b_in = nc.dram_tensor("be2base", [128, 2], f32, kind="ExternalInput")
    out_t = nc.dram_tensor("out", [OUT, NLOC], f32, kind="ExternalOutput")

    with tile.TileContext(nc) as tc:
        with (
            tc.tile_pool(name="const", bufs=1) as cpool,
            tc.tile_pool(name="work", bufs=2) as wpool,
            tc.tile_pool(name="slabp", bufs=4) as slabpool,
            tc.tile_pool(name="selp", bufs=4) as selpool,
            tc.tile_pool(name="stage", bufs=3) as stpool,
            tc.tile_pool(name="psum_sel", bufs=2, space="PSUM") as ps_sel,
            tc.tile_pool(name="psum_xf", bufs=2, space="PSUM") as ps_xf,
            tc.tile_pool(name="psum_misc", bufs=1, space="PSUM") as ps_misc,
            tc.tile_pool(name="dram", bufs=1, space="DRAM") as dpool,
            tc.tile_pool(name="dramsh", bufs=1, space="DRAM") as shpool,
        ):
            # constants / weights
            ident = cpool.tile([128, 128], f32)
            make_identity(nc, ident[:])
            ident16 = cpool.tile([128, 128], f16)
            nc.vector.tensor_copy(ident16[:], ident[:])
            iota_i = cpool.tile([128, N_GRID], dt.int32)
            nc.gpsimd.iota(iota_i[:], pattern=[[1, N_GRID]], base=0,
                           channel_multiplier=0)
            iota_f = cpool.tile([128, N_GRID], f32)
            nc.vector.tensor_copy(iota_f[:], iota_i[:])
            ones16 = cpool.tile([1, 128], f16)
            nc.vector.memset(ones16[:], 1.0)

            def load_const(t_in, shape, re=None, tag=None, cdt=f32):
                t = cpool.tile(shape, cdt, tag=tag)
                nc.sync.dma_start(t[:], t_in[:] if re is None else t_in[:].rearrange(re))
                return t

            # dense weights: [hilo, 128, 2, D] -> sbuf [128, hilo, 2, D]
            w_in_sb = load_const(w_in_in, [128, 2, 2, D], "h p k d -> p h k d",
                                 tag="w_in", cdt=f16)
            w_root_sb = load_const(w_root_in, [128, 2, 2, D],
                                   "h p k d -> p h k d", tag="w_root", cdt=f16)
            w_rel_sb = load_const(w_rel_in, [128, R, 2, D], "r p k d -> p r k d",
                                  tag="w_rel")
            b_in_sb = load_const(b_in_in, [128, 2], tag="b_in")
            b_rg_sb = load_const(b_rgcn_in, [128, 2], tag="b_rg")
            wgd_sb = load_const(wg_in, [128, 2, 2, 1], "h p k d -> p h k d",
                                tag="wgd", cdt=f16)
            we1_sb = load_const(we1_in, [128, NE, 2, D], "e p k d -> p e k d",
                                tag="we1", cdt=f16)
            be1_sb = load_const(be1_in, [128, NE, 2], "e p k -> p e k", tag="be1")
            we2_sb = load_const(we2_in, [128, NE, 2, OUT], "e p k d -> p e k d",
                                tag="we2", cdt=f16)
            be2d_sb = cpool.tile([1, 2, 2, 128], f16)
            nc.sync.dma_start(be2d_sb[:], be2d_in[:])
            be2b_sb = load_const(be2b_in, [128, 2], tag="be2b")

            seg_sb = cpool.tile([128, NSLOT], f32)
            nc.sync.dma_start(seg_sb[:], seg_in[:])
            idx_sb = cpool.tile([128, IDXC], dt.int16)
            nc.sync.dma_start(idx_sb[:], idx16_in[:])

            # DRAM staging: fp16 hi/lo pairs for inter-layer x
            xT1h = dpool.tile([128, 2, NLOC], f16)
            xT1l = dpool.tile([128, 2, NLOC], f16)
            xT2h = dpool.tile([128, 2, NLOC], f16)
            xT2l = dpool.tile([128, 2, NLOC], f16)
            xloc1 = dpool.tile([NLOC, TW], f16)
            xloc2 = dpool.tile([NLOC, TW], f16)
            xfulls = []  # [rep][layer][chunk]
            for _r in range(REPS):
                per_layer = []
                for li in (1, 2):
                    per_layer.append([
                        shpool.tile([NCORES * csz[k], TW], f16,
                                    addr_space="Shared",
                                    name="xf%d_%d_%d" % (li, _r, k))
                        for k in range(agc)
                    ])
                xfulls.append(per_layer)

            def win_sizes(w):
                wb = w * WIN
                return wb, min(WIN, NLOC - wb)

            def mm3(ps, w_sb, xh, xlo, mc, nw, start, stop):
                """3-term hi/lo dense matmul: out[mc] += W.T @ x (exact).

                w_sb: [128, hilo, 2, D] fp16; xh/xlo: [128, 2, nw] fp16.
                """
                for kc in range(2):
                    first = start and kc == 0
                    nc.tensor.matmul(
                        ps, w_sb[:, 0, kc, mc * 128:(mc + 1) * 128],
                        xh[:, kc, :nw], start=first, stop=False)
                    nc.tensor.matmul(
                        ps, w_sb[:, 0, kc, mc * 128:(mc + 1) * 128],
                        xlo[:, kc, :nw], start=False, stop=False)
                    last = stop and kc == 1
                    nc.tensor.matmul(
                        ps, w_sb[:, 1, kc, mc * 128:(mc + 1) * 128],
                        xh[:, kc, :nw], start=False, stop=last)

            def export_window(xh, xlo, wb, nw, xloc):
                # fp16 transpose (128, 2, nw) hi+lo -> node-major rows of xloc.
                # One packed PSUM tile = 1 bank; the two halves double-buffer
                # the per-128-node blocks manually.
                pst2 = ps_misc.tile([128, 2, TW], f16, space="PSUM",
                                    tag="misc16")
                nb = 0
                blk = 0
                while nb < nw:
                    bs = min(128, nw - nb)
                    stg = stpool.tile([128, TW], f16, tag="stage")
                    pst = pst2[:, blk % 2, :]
                    for mc in range(2):
                        nc.tensor.transpose(pst[:bs, mc * 128:(mc + 1) * 128],
                                            xh[:, mc, nb:nb + bs], ident16[:])
                        nc.tensor.transpose(pst[:bs, D + mc * 128:D + (mc + 1) * 128],
                                            xlo[:, mc, nb:nb + bs], ident16[:])
                    nc.scalar.activation(stg[:bs, :], pst[:bs, :], ACT.Copy)
                    nc.sync.dma_start(xloc[wb + nb: wb + nb + bs, :], stg[:bs, :])
                    nb += bs
                    blk += 1

            for _rep in range(REPS):
                xfull1, xfull2 = xfulls[_rep]
                ag_state1, ag_state2 = [0], [0]

                def ag_emit(xloc, xflist, rows_done, state):
                    while state[0] < agc and bounds[state[0] + 1] <= rows_done:
                        k = state[0]
                        if not SKIP_COLL:
                            nc.gpsimd.collective_compute(
                                "AllGather", mybir.AluOpType.bypass,
                                replica_groups=[list(range(NCORES))],
                                ins=[xloc[bounds[k]:bounds[k + 1], :].opt()],
                                outs=[xflist[k][:].opt()])
                        state[0] += 1

                # ------------ layer 0: x1 = selu(x_cat @ W_in + b_in) --------
                for w in range(NWIN):
                    wb, nw = win_sizes(w)
                    xwh = wpool.tile([128, 2, WIN], f16, tag="xwh")
                    xwl = wpool.tile([128, 2, WIN], f16, tag="xwl")
                    nc.sync.dma_start(xwh[:, :, :nw], xcat_h_in[:, :, wb:wb + nw])
                    nc.sync.dma_start(xwl[:, :, :nw], xcat_l_in[:, :, wb:wb + nw])
                    xoh = wpool.tile([128, 2, WIN], f16, tag="xoh")
                    xol = wpool.tile([128, 2, WIN], f16, tag="xol")
                    for mc in range(2):
                        ps = ps_xf.tile([128, WIN], f32, space="PSUM", tag="xf")
                        mm3(ps[:, :nw], w_in_sb, xwh, xwl, mc, nw,
                            start=True, stop=True)
                        pos = wpool.tile([128, WIN], f32, tag="selu_pos")
                        nc.vector.tensor_scalar(
                            out=pos[:, :nw], in0=ps[:, :nw],
                            scalar1=b_in_sb[:, mc:mc + 1], scalar2=0.0,
                            op0=Alu.add, op1=Alu.max)
                        neg = wpool.tile([128, WIN], f32, tag="selu_neg")
                        nc.vector.tensor_scalar(
                            out=neg[:, :nw], in0=ps[:, :nw],
                            scalar1=b_in_sb[:, mc:mc + 1], scalar2=0.0,
                            op0=Alu.add, op1=Alu.min)
                        e = wpool.tile([128, WIN], f32, tag="selu_e")
                        nc.scalar.activation(e[:, :nw], neg[:, :nw], ACT.Exp)
                        sa = SELU_SCALE * SELU_ALPHA
                        nc.vector.tensor_scalar(
                            out=e[:, :nw], in0=e[:, :nw], scalar1=sa, scalar2=sa,
                            op0=Alu.mult, op1=Alu.subtract)
                        nc.vector.tensor_scalar(
                            out=pos[:, :nw], in0=pos[:, :nw],
                            scalar1=SELU_SCALE, scalar2=None, op0=Alu.mult)
                        xo = wpool.tile([128, WIN], f32, tag="xo32")
                        nc.vector.tensor_tensor(
                            out=xo[:, :nw], in0=pos[:, :nw], in1=e[:, :nw],
                            op=Alu.add)
                        nc.scalar.activation(xoh[:, mc, :nw], xo[:, :nw],
                                             ACT.Copy)
                        nc.vector.tensor_tensor(
                            out=xol[:, mc, :nw], in0=xo[:, :nw],
                            in1=xoh[:, mc, :nw], op=Alu.subtract)
                    nc.sync.dma_start(xT1h[:, :, wb:wb + nw], xoh[:, :, :nw])
                    nc.sync.dma_start(xT1l[:, :, wb:wb + nw], xol[:, :, :nw])
                    export_window(xoh, xol, wb, nw, xloc1)
                    if AG_OVERLAP:
                        ag_emit(xloc1, xfull1, wb + nw, ag_state1)
                ag_emit(xloc1, xfull1, NLOC, ag_state1)

                # ------------ rgcn layers ------------
                def rgcn_layer(xfull, xTh_cur, xTl_cur, xTh_next, xTl_next,
                               xloc_next, li, moe_fn=None, xfull_next=None,
                               ag_state=None):
                    for w in range(NWIN):
                        wb, nw = win_sizes(w)
                        # load x and issue the root matmuls first: they only
                        # need xT, so the PE has work while gathers land
                        xwh = wpool.tile([128, 2, WIN], f16, tag="xwh")
                        xwl = wpool.tile([128, 2, WIN], f16, tag="xwl")
                        nc.sync.dma_start(xwh[:, :, :nw],
                                          xTh_cur[:, :, wb:wb + nw])
                        nc.sync.dma_start(xwl[:, :, :nw],
                                          xTl_cur[:, :, wb:wb + nw])
                        ps_mc = []
                        for mc in range(2):
                            ps = ps_xf.tile([128, WIN], f32, space="PSUM",
                                            tag="xf")
                            mm3(ps[:, :nw], w_root_sb, xwh, xwl, mc, nw,
                                start=True, stop=False)
                            ps_mc.append(ps)
                        s_tiles = {}
                        for r in range(R):
                            wi = r * NWIN + w
                            cursor = plan["win_slot_start"][wi]
                            kcounts = plan["win_slot_chunks"][wi]
                            if SLOT_CAP is not None:
                                kcounts = [min(kcounts[0], SLOT_CAP)] + \
                                    [0] * (len(kcounts) - 1)
                            nslots_w = sum(kcounts)
                            slab = slabpool.tile([128, MAX_SLOTS, TW], f16,
                                                 tag="slab")
                            off = 0
                            for k in range(agc):
                                ck, ni = plan["win_idx_cols"][wi][k]
                                nsl = kcounts[k]
                                ni = min(ni, nsl * CHUNK)
                                if ni and not SKIP_GATHER:
                                    nc.gpsimd.dma_gather(
                                        out_ap=slab[:, off:off + nsl, :],
                                        in_ap=xfull[k][:],
                                        idxs_ap=idx_sb[:, ck:ck + ni // 16],
                                        num_idxs=ni, num_idxs_reg=ni,
                                        elem_size=TW, single_packet=False,
                                        queue_num=(r * agc + k) % N_SWDGE_Q)
                                off += nsl
                            ps0 = ps_sel.tile([128, WIN], f32, space="PSUM",
                                              tag="sel0")
                            ps1 = ps_sel.tile([128, WIN], f32, space="PSUM",
                                              tag="sel1")
                            for s in range(nslots_w):
                                sl = cursor + s
                                gi = int(slot_group[sl])
                                gb_in_win = groups[gi][2] - wb
                                gs = groups[gi][3]
                                sel = selpool.tile([128, N_GRID], f16, tag="sel")
                                nc.vector.tensor_scalar(
                                    out=sel[:, :gs], in0=iota_f[:, :gs],
                                    scalar1=seg_sb[:, sl:sl + 1], scalar2=None,
                                    op0=Alu.is_equal)
                                cols = slice(gb_in_win, gb_in_win + gs)
                                last = s == nslots_w - 1
                                nc.tensor.matmul(
                                    ps0[:, cols], slab[:, s, 0:128],
                                    sel[:, :gs], start=(s == 0), stop=False)
                                nc.tensor.matmul(
                                    ps0[:, cols], slab[:, s, 256:384],
                                    sel[:, :gs], start=False, stop=last)
                                nc.tensor.matmul(
                                    ps1[:, cols], slab[:, s, 128:256],
                                    sel[:, :gs], start=(s == 0), stop=False)
                                nc.tensor.matmul(
                                    ps1[:, cols], slab[:, s, 384:512],
                                    sel[:, :gs], start=False, stop=last)
                            # broadcast cnt_inv to 128 rows: two fp16 rank-1
                            # matmuls (hi+lo) accumulate the exact f32 value
                            ci16 = wpool.tile([1, 2, WIN], f16, tag="ci_row")
                            nc.sync.dma_start(ci16[:], cntinv_in[0, wi:wi + 1])
                            cib = ps_misc.tile([128, WIN], f32, space="PSUM",
                                               tag="misc")
                            nc.tensor.matmul(cib[:, :nw], ones16[:],
                                             ci16[0:1, 0, :nw],
                                             start=True, stop=False)
                            nc.tensor.matmul(cib[:, :nw], ones16[:],
                                             ci16[0:1, 1, :nw],
                                             start=False, stop=True)
                            ci = wpool.tile([128, WIN], f32, tag="cntinv")
                            nc.scalar.activation(ci[:, :nw], cib[:, :nw], ACT.Copy)
                            s0 = wpool.tile([128, WIN], f32, tag="s0_%d" % r)
                            s1 = wpool.tile([128, WIN], f32, tag="s1_%d" % r)
                            nc.vector.tensor_tensor(out=s0[:, :nw], in0=ps0[:, :nw],
                                                    in1=ci[:, :nw], op=Alu.mult)
                            nc.vector.tensor_tensor(out=s1[:, :nw], in0=ps1[:, :nw],
                                                    in1=ci[:, :nw], op=Alu.mult)
                            s_tiles[r] = (s0, s1)

                        xoh = wpool.tile([128, 2, WIN], f16, tag="xoh")
                        xol = wpool.tile([128, 2, WIN], f16, tag="xol")
                        for mc in range(2):
                            ps = ps_mc[mc]
                            for r in range(R):
                                for kc in range(2):
                                    st = s_tiles[r][kc]
                                    nc.tensor.matmul(
                                        ps[:, :nw],
                                        w_rel_sb[:, r, kc, mc * 128:(mc + 1) * 128],
                                        st[:, :nw],
                                        start=False, stop=(r == R - 1 and kc == 1))
                            # x = ps + b, split to fp16 hi/lo (Identity
                            # accepts an AP bias; Copy does not)
                            nc.scalar.activation(xoh[:, mc, :nw], ps[:, :nw],
                                                 ACT.Identity,
                                                 bias=b_rg_sb[:, mc:mc + 1])
                            xf = wpool.tile([128, WIN], f32, tag="xf32")
                            nc.vector.tensor_scalar(
                                out=xf[:, :nw], in0=ps[:, :nw],
                                scalar1=b_rg_sb[:, mc:mc + 1], scalar2=None,
                                op0=Alu.add)
                            nc.vector.tensor_tensor(
                                out=xol[:, mc, :nw], in0=xf[:, :nw],
                                in1=xoh[:, mc, :nw], op=Alu.subtract)
                        if xTh_next is not None:
                            nc.sync.dma_start(xTh_next[:, :, wb:wb + nw],
                                              xoh[:, :, :nw])
                            nc.sync.dma_start(xTl_next[:, :, wb:wb + nw],
                                              xol[:, :, :nw])
                        if xloc_next is not None:
                            export_window(xoh, xol, wb, nw, xloc_next)
                            if xfull_next is not None and AG_OVERLAP:
                                ag_emit(xloc_next, xfull_next, wb + nw, ag_state)
                        if moe_fn is not None:
                            moe_fn(xoh, xol, wb, nw)
                    if xfull_next is not None:
                        ag_emit(xloc_next, xfull_next, NLOC, ag_state)

                # ------------ MoE (fused into layer-2 windows) ------------
                def moe_window(xh, xlo, wb, nw):
                    psl = ps_misc.tile([128, WIN], f32, space="PSUM", tag="misc")
                    for kc in range(2):
                        nc.tensor.matmul(psl[:1, :nw], wgd_sb[:, 0, kc, :],
                                         xh[:, kc, :nw], start=(kc == 0),
                                         stop=False)
                        nc.tensor.matmul(psl[:1, :nw], wgd_sb[:, 0, kc, :],
                                         xlo[:, kc, :nw], start=False, stop=False)
                        nc.tensor.matmul(psl[:1, :nw], wgd_sb[:, 1, kc, :],
                                         xh[:, kc, :nw], start=False,
                                         stop=(kc == 1))
                    g_row = wpool.tile([1, WIN], f32, tag="grow")
                    nc.vector.tensor_scalar(out=g_row[:, :nw], in0=psl[:1, :nw],
                                            scalar1=0.0, scalar2=None,
                                            op0=Alu.is_ge)
                    g16 = wpool.tile([1, WIN], f16, tag="g16")
                    nc.scalar.activation(g16[:, :nw], g_row[:, :nw], ACT.Copy)
                    psb = ps_misc.tile([128, WIN], f32, space="PSUM", tag="misc")
                    nc.tensor.matmul(psb[:, :nw], ones16[:], g16[:, :nw],
                                     start=True, stop=True)
                    gb = wpool.tile([128, WIN], f32, tag="gb_sb")
                    nc.scalar.activation(gb[:, :nw], psb[:, :nw], ACT.Copy)
                    ginv = wpool.tile([128, WIN], f32, tag="ginv")
                    nc.vector.tensor_scalar(out=ginv[:, :nw], in0=gb[:, :nw],
                                            scalar1=-1.0, scalar2=1.0,
                                            op0=Alu.mult, op1=Alu.add)

                    h1g = {}
                    for e in range(NE):
                        for mc in range(2):
                            psh = ps_xf.tile([128, WIN], f32, space="PSUM",
                                             tag="xf")
                            for kc in range(2):
                                nc.tensor.matmul(
                                    psh[:, :nw],
                                    we1_sb[:, e, kc, mc * 128:(mc + 1) * 128],
                                    xh[:, kc, :nw],
                                    start=(kc == 0), stop=(kc == 1))
                            h = wpool.tile([128, WIN], f16,
                                           tag="h1_%d_%d" % (e, mc))
                            nc.scalar.activation(
                                h[:, :nw], psh[:, :nw], ACT.Lrelu,
                                bias=be1_sb[:, e, mc:mc + 1], alpha=NEG_SLOPE)
                            gt = gb if e == 0 else ginv
                            nc.vector.tensor_tensor(out=h[:, :nw], in0=h[:, :nw],
                                                    in1=gt[:, :nw], op=Alu.mult)
                            h1g[(e, mc)] = h
                    for mc in range(2):
                        psy = ps_xf.tile([128, WIN], f32, space="PSUM", tag="xf")
                        first = True
                        for e in range(NE):
                            for kc in range(2):
                                nc.tensor.matmul(
                                    psy[:, :nw],
                                    we2_sb[:, e, kc, mc * 128:(mc + 1) * 128],
                                    h1g[(e, kc)][:, :nw],
                                    start=first, stop=False)
                                first = False
                        # + (be2[0]-be2[1]) (x) g  via hi/lo rank-1 fp16
                        nc.tensor.matmul(psy[:, :nw], be2d_sb[0:1, 0, mc, :],
                                         g16[:, :nw], start=False, stop=False)
                        nc.tensor.matmul(psy[:, :nw], be2d_sb[0:1, 1, mc, :],
                                         g16[:, :nw], start=False, stop=True)
                        yt = wpool.tile([128, WIN], f32, tag="yt")
                        nc.scalar.activation(yt[:, :nw], psy[:, :nw],
                                             ACT.Identity,
                                             bias=be2b_sb[:, mc:mc + 1])
                        nc.sync.dma_start(out_t[mc * 128:(mc + 1) * 128,
                                                wb:wb + nw],
                                          yt[:, :nw])

                rgcn_layer(xfull1, xT1h, xT1l, xT2h, xT2l, xloc2, 1,
                           xfull_next=xfull2, ag_state=ag_state2)
                rgcn_layer(xfull2, xT2h, xT2l, None, None, None, 2,
                           moe_fn=moe_window)

    nc.compile()
    return nc


# ----------------------------------------------------------------------------
# entry point
# ----------------------------------------------------------------------------

def _hilo(a):
    a = np.asarray(a, np.float32)
    hi = a.astype(np.float16)
    lo = (a - hi.astype(np.float32)).astype(np.float16)
    return hi, lo


def kernel(des, tweet, num_prop, cat_prop, edge_index, edge_type,
           W_in, b_in, W_rel, W_root, b_rgcn, w_gate, We1, be1, We2, be2):
    from concourse.bass_utils import run_bass_kernel_spmd

    x_cat = np.concatenate(
        [np.asarray(des), np.asarray(tweet), np.asarray(num_prop),
         np.asarray(cat_prop)], axis=1).astype(np.float32)

    plan, data = build_plan(np.asarray(edge_index), np.asarray(edge_type))
    nc = build_nc(plan)

    def wmat(w):
        w = np.asarray(w, np.float32)
        return np.ascontiguousarray(w.reshape(2, 128, w.shape[1]).transpose(1, 0, 2))

    def wmat2(w):
        return np.stack(_hilo(wmat(w)))  # (2=hi/lo, 128, 2, D)

    def bvec(b):
        return np.ascontiguousarray(np.asarray(b, np.float32).reshape(2, 128).T)

    w_rel_h = np.stack([wmat(np.asarray(W_rel)[r]) for r in range(R)])
    we1_h = np.stack([wmat(np.asarray(We1)[e]).astype(np.float16)
                      for e in range(NE)])
    be1_h = np.stack([bvec(np.asarray(be1)[e]) for e in range(NE)])
    we2_h = np.stack([wmat(np.asarray(We2)[e]).astype(np.float16)
                      for e in range(NE)])
    be2 = np.asarray(be2, np.float32)
    be2d_hi, be2d_lo = _hilo((be2[0] - be2[1]).reshape(1, 2, 128))
    be2d = np.stack([be2d_hi, be2d_lo], axis=1)  # (1, 2hilo, 2mc, 128)
    be2b = bvec(be2[1])

    wg = np.asarray(w_gate, np.float32)
    wgd2 = np.stack(_hilo(wmat(wg[:, 0:1] - wg[:, 1:2])))

    in_maps = []
    for c in range(NCORES):
        xc = x_cat[c * NLOC:(c + 1) * NLOC]
        xcatT = np.ascontiguousarray(xc.T.reshape(2, 128, NLOC).transpose(1, 0, 2))
        xch, xcl = _hilo(xcatT)
        in_maps.append({
            "xcath": xch,
            "xcatl": xcl,
            "idx16": data["idx16"][c],
            "segloc": data["seg_all"][c],
            "cntinv16": data["cntinv16"][c][None],
            "w_in2": wmat2(W_in), "w_root2": wmat2(W_root), "w_rel": w_rel_h,
            "b_in": bvec(b_in), "b_rgcn": bvec(b_rgcn),
            "wgate2": wgd2,
            "we1": we1_h, "be1": be1_h,
            "we2": we2_h, "be2diff": be2d, "be2base": be2b,
        })

    res = run_bass_kernel_spmd(nc, in_maps, core_ids=list(range(NCORES)))
    global last_nc, last_in_maps
    last_nc, last_in_maps = nc, in_maps
    y = np.concatenate([res.results[c]["out"].T for c in range(NCORES)], axis=0)
    return y.astype(np.float32)


last_nc = None
last_in_maps = None


# revision 30
# speedup vs baseline: 88.4887x; 1.7604x over previous
"""BotRGCN + MoE (top-1 of 2) Trainium2 Bass kernel, 8-core SPMD. v3.

v3 over v2 (PE was the bottleneck at ~789us/core in TimelineSim):
  - All dense x-pipeline matmuls (W_in, W_root, w_gate) run as exact fp16
    hi/lo 3-term products (Whi@xhi + Whi@xlo + Wlo@xhi, ~2^-22 rel error)
    instead of fp32 (3 cyc/col vs 4). x is carried as an fp16 hi/lo pair
    end-to-end; the pair is also what the gather table already needed.
  - MoE expert matmuls in plain fp16 (post-gate path, 5e-4 rel is fine).
  - cnt_inv broadcast via two fp16 rank-1 matmuls (hi+lo, exact); MoE gate
    broadcast via fp16 (g is exactly 0/1); expert-bias column via a single
    hi/lo rank-1 of (be2[0]-be2[1]) plus a per-partition Act bias.
  - xT inter-layer staging in DRAM as fp16 hi/lo pairs (half the bytes).
  - Export transposes run on fp16 (1 cyc/row) with one fused stage copy into
    a packed 1-bank PSUM tile whose halves double-buffer the node blocks.
  - Per window the root matmuls are issued before the gather-dependent
    selector work so the PE has work while slab gathers land.
  - 4 SWDGE queues + 4 slab buffers: HW ablations show the SWDGE gather
    stream (~63MB/layer/core of random 1KB rows) is the bottleneck (~1.2ms
    of the ~1.74ms body); deeper gather pipelining buys ~100us.

Self-contained: hardcodes shapes; imports only installed packages.
"""

import numpy as np

N = 50000
E = 400000
D = 256
R = 2
NE = 2
OUT = 256
NCORES = 8
NLOC = N // NCORES  # 6250
WIN = 512
NWIN = (NLOC + WIN - 1) // WIN  # 13
CHUNK = 128
N_GRID = 128

# config
AG_CHUNKS = 2         # table chunk tensors / collectives per layer (>=2)
AG_OVERLAP = True     # emit chunk AllGathers inline with producing windows
N_SWDGE_Q = 4         # swdge queues for gathers
REPS = 1              # repeat whole body (marginal-cost timing)
SKIP_COLL = False     # timing-ablation only: skip collectives
SKIP_GATHER = False   # timing-ablation only: skip dma_gathers too
SLOT_CAP = None       # timing-ablation only: cap selector slots per (win,rel)

SELU_SCALE = 1.0507009873554805
SELU_ALPHA = 1.6732632423543772
NEG_SLOPE = 0.01

TW = 2 * D  # table row width in fp16 elements (hi 256 | lo 256)


def _chunk_bounds(agc):
    # WIN-aligned, slightly unbalanced splits minimize per-group ceil padding
    table = {
        2: [0, 3584, NLOC],
        3: [0, 2048, 4096, NLOC],
        4: [0, 1536, 3072, 4608, NLOC],
    }
    if agc in table:
        return table[agc]
    base = max(WIN, (NLOC // agc) // WIN * WIN)
    return [min(k * base, NLOC) for k in range(agc)] + [NLOC]


def _wrap_idx(idx):
    """int16 index list (len multiple of 16) -> (128, len/16) wrapped+replicated."""
    n = len(idx)
    w = idx.reshape(n // 16, 16).T.astype(np.int16)
    return np.tile(w, (8, 1))


# ----------------------------------------------------------------------------
# host-side planning
# ----------------------------------------------------------------------------

def build_plan(edge_index, edge_type):
    agc = AG_CHUNKS
    bounds = _chunk_bounds(agc)
    csz = [bounds[k + 1] - bounds[k] for k in range(agc)]

    src = np.asarray(edge_index[0], dtype=np.int64)
    dst = np.asarray(edge_index[1], dtype=np.int64)
    rel = np.asarray(edge_type, dtype=np.int64)

    core = dst // NLOC
    seg = rel * NLOC + (dst % NLOC)
    NSEG = R * NLOC

    cnt = np.bincount((core * NSEG + seg).astype(np.int64),
                      minlength=NCORES * NSEG).reshape(NCORES, NSEG)

    # chunk + row-in-chunk-tensor of each edge's source node
    src_c = src // NLOC
    src_j = src % NLOC
    src_k = np.searchsorted(bounds, src_j, side="right") - 1
    csz_arr = np.asarray(csz)
    b_arr = np.asarray(bounds[:-1])
    src_row = src_c * csz_arr[src_k] + (src_j - b_arr[src_k])

    # groups: (r, wbase, gbase, gsize, win_index). Per (win, rel) a final
    # window-wide overflow group absorbs each core's spill past the base
    # slots (base slots are sized by the min over cores, so they are always
    # completely full — padding only exists in the overflow slots).
    groups = []
    ovf_of_win = {}
    wi = 0
    for r in range(R):
        for w in range(NWIN):
            wb = w * WIN
            nw = min(WIN, NLOC - wb)
            g0 = 0
            while g0 < nw:
                gs = min(N_GRID, nw - g0)
                groups.append((r, wb, wb + g0, gs, wi))
                g0 += gs
            ovf_of_win[wi] = len(groups)
            groups.append((r, wb, wb, nw, wi))
            wi += 1
    n_windows_total = wi
    NG = len(groups)

    win_groups = [[] for _ in range(n_windows_total)]
    for gi, g in enumerate(groups):
        win_groups[g[4]].append(gi)

    # per-core edge lists sorted by (seg, chunk)
    per_core = []
    for c in range(NCORES):
        m = core == c
        s_seg, s_row, s_k = seg[m], src_row[m], src_k[m]
        o = np.lexsort((s_k, s_seg))
        per_core.append((s_seg[o], s_row[o], s_k[o]))

    # base slots per (regular group, chunk) = min over cores count//CHUNK
    # (always full); overflow slots per (win, chunk) = max over cores of
    # ceil(total spill / CHUNK)
    n_gk = np.zeros((NG, agc), np.int64)
    cnt_gk = np.full((NCORES, NG, agc), 0, np.int64)
    core_group_edges = []  # [core][gi][k] -> (rows, seg_local) sorted by row
    for c in range(NCORES):
        s_seg, s_row, s_k = per_core[c]
        lst = []
        for gi, (r, wb, gb, gs, gw) in enumerate(groups):
            if gi == ovf_of_win.get(gw):
                lst.append([None] * agc)
                continue
            lo_b = np.searchsorted(s_seg, r * NLOC + gb)
            hi_b = np.searchsorted(s_seg, r * NLOC + gb + gs)
            rows, gg, kk = (s_row[lo_b:hi_b], s_seg[lo_b:hi_b] - (r * NLOC + gb),
                            s_k[lo_b:hi_b])
            per_k = []
            for k in range(agc):
                mk = kk == k
                rk, gk = rows[mk], gg[mk]
                # ascending source rows: the gather descriptors then read
                # increasing HBM addresses (row-buffer locality)
                o2 = np.argsort(rk, kind="stable")
                per_k.append((rk[o2], gk[o2]))
                cnt_gk[c, gi, k] = int(mk.sum())
            lst.append(per_k)
        core_group_edges.append(lst)

    for gi, (r, wb, gb, gs, gw) in enumerate(groups):
        if gi == ovf_of_win.get(gw):
            continue
        for k in range(agc):
            n_gk[gi, k] = int(cnt_gk[:, gi, k].min()) // CHUNK
    for w in range(n_windows_total):
        ogi = ovf_of_win[w]
        regs = [gi for gi in win_groups[w] if gi != ogi]
        for k in range(agc):
            spill = np.zeros(NCORES, np.int64)
            for gi in regs:
                spill += cnt_gk[:, gi, k] - n_gk[gi, k] * CHUNK
            n_gk[ogi, k] = int(-(-spill.max() // CHUNK))
        # every (win, rel) needs >=1 slot so the PSUM bank gets cleared
        if sum(int(n_gk[gi, k]) for gi in win_groups[w] for k in range(agc)) == 0:
            n_gk[ogi, 0] = 1

    # slot layout per (win, rel): chunk-major, then group order
    slot_group = []
    win_slot_chunks = []  # per win: list over k of slot count
    win_slot_start = []
    for w in range(n_windows_total):
        start = len(slot_group)
        per_k_counts = []
        for k in range(agc):
            nk = 0
            for gi in win_groups[w]:
                slot_group.extend([gi] * int(n_gk[gi, k]))
                nk += int(n_gk[gi, k])
            per_k_counts.append(nk)
        win_slot_start.append(start)
        win_slot_chunks.append(per_k_counts)
    n_slots = len(slot_group)

    idx_all = np.zeros((NCORES, n_slots, CHUNK), np.int16)
    seg_all = np.full((NCORES, CHUNK, n_slots), -1.0, np.float32)
    for c in range(NCORES):
        for w in range(n_windows_total):
            ogi = ovf_of_win[w]
            wb = groups[ogi][1]
            cursor = win_slot_start[w]
            for k in range(agc):
                sp_rows, sp_segs = [], []
                for gi in win_groups[w]:
                    nsl = int(n_gk[gi, k])
                    if gi == ogi:
                        rr = np.concatenate(sp_rows) if sp_rows else \
                            np.zeros(0, np.int64)
                        gp = np.concatenate(sp_segs) if sp_segs else \
                            np.zeros(0, np.int64)
                        o3 = np.argsort(rr, kind="stable")
                        rr, gp = rr[o3], gp[o3]
                    else:
                        rows, gg = core_group_edges[c][gi][k]
                        nb = nsl * CHUNK
                        rr, gp = rows[:nb], gg[:nb]
                        gb_in_win = groups[gi][2] - wb
                        sp_rows.append(rows[nb:])
                        sp_segs.append(gg[nb:] + gb_in_win)
                    if nsl == 0:
                        continue
                    ne = len(rr)
                    pad = nsl * CHUNK - ne
                    assert pad >= 0, (w, k, gi, ne, nsl)
                    rr = np.concatenate([rr, np.zeros(pad, np.int64)])
                    gp = np.concatenate([gp, np.full(pad, -1, np.int64)])
                    for s in range(nsl):
                        sl = cursor + s
                        idx_all[c, sl] = rr[s * CHUNK:(s + 1) * CHUNK].astype(np.int16)
                        seg_all[c, :, sl] = gp[s * CHUNK:(s + 1) * CHUNK].astype(np.float32)
                    cursor += nsl

    # idx16 column layout: per (win, rel), per chunk
    win_idx_cols = []  # per win: [(col, ni), ...] per chunk
    col = 0
    for w in range(n_windows_total):
        entry = []
        for k in range(agc):
            ni = win_slot_chunks[w][k] * CHUNK
            entry.append((col, ni))
            col += ni // 16
        win_idx_cols.append(entry)
    tot_cols = col

    idx16 = []
    for c in range(NCORES):
        buf = np.zeros((128, tot_cols), np.int16)
        for w in range(n_windows_total):
            cursor = win_slot_start[w]
            for k in range(agc):
                nsl = win_slot_chunks[w][k]
                ck, ni = win_idx_cols[w][k]
                if ni:
                    buf[:, ck:ck + ni // 16] = _wrap_idx(
                        idx_all[c, cursor:cursor + nsl].reshape(-1))
                cursor += nsl
        idx16.append(buf)

    # cnt_inv rows, fp16 hi + fp16 residual (exact through psum accumulate)
    cntinv = np.ones((NCORES, n_windows_total, WIN), np.float64)
    for c in range(NCORES):
        for r in range(R):
            for w in range(NWIN):
                wb = w * WIN
                nw = min(WIN, NLOC - wb)
                cc = cnt[c, r * NLOC + wb: r * NLOC + wb + nw]
                cntinv[c, r * NWIN + w, :nw] = 1.0 / np.maximum(cc, 1)
    ci32 = cntinv.astype(np.float32)
    ci_hi = ci32.astype(np.float16)
    ci_lo = (ci32 - ci_hi.astype(np.float32)).astype(np.float16)
    cntinv16 = np.stack([ci_hi, ci_lo], axis=2)  # (NCORES, NSEGW, 2, WIN)

    plan = dict(
        agc=agc, bounds=bounds, csz=csz,
        groups=groups,
        slot_group=np.array(slot_group, np.int64),
        win_groups=win_groups,
        win_slot_start=win_slot_start,
        win_slot_chunks=win_slot_chunks,
        win_idx_cols=win_idx_cols,
        n_slots=n_slots,
        idx_cols=tot_cols,
        n_windows_total=n_windows_total,
        max_slots=max(sum(cc) for cc in win_slot_chunks),
    )
    data = dict(idx16=idx16, seg_all=seg_all, cntinv16=cntinv16)
    return plan, data


# ----------------------------------------------------------------------------
# bass program
# ----------------------------------------------------------------------------

def build_nc(plan):
    import concourse.mybir as mybir
    import concourse.tile as tile
    from concourse import bacc
    from concourse.masks import make_identity

    dt = mybir.dt
    f32 = dt.float32
    f16 = dt.float16
    Alu = mybir.AluOpType
    ACT = mybir.ActivationFunctionType

    agc = plan["agc"]
    bounds = plan["bounds"]
    csz = plan["csz"]
    NSEGW = plan["n_windows_total"]
    NSLOT = plan["n_slots"]
    IDXC = plan["idx_cols"]
    MAX_SLOTS = plan["max_slots"]
    groups = plan["groups"]
    slot_group = plan["slot_group"]

    nc = bacc.Bacc(None, num_devices=NCORES, num_swdge_queues=N_SWDGE_Q)

    xcat_h_in = nc.dram_tensor("xcath", [128, 2, NLOC], f16, kind="ExternalInput")
    xcat_l_in = nc.dram_tensor("xcatl", [128, 2, NLOC], f16, kind="ExternalInput")
    idx16_in = nc.dram_tensor("idx16", [128, IDXC], dt.int16, kind="ExternalInput")
    seg_in = nc.dram_tensor("segloc", [128, NSLOT], f32, kind="ExternalInput")
    cntinv_in = nc.dram_tensor("cntinv16", [1, NSEGW, 2, WIN], f16,
                               kind="ExternalInput")
    w_in_in = nc.dram_tensor("w_in2", [2, 128, 2, D], f16, kind="ExternalInput")
    w_root_in = nc.dram_tensor("w_root2", [2, 128, 2, D], f16,
                               kind="ExternalInput")
    w_rel_in = nc.dram_tensor("w_rel", [R, 128, 2, D], f32, kind="ExternalInput")
    b_in_in = nc.dram_tensor("b_in", [128, 2], f32, kind="ExternalInput")
    b_rgcn_in = nc.dram_tensor("b_rgcn", [128, 2], f32, kind="ExternalInput")
    wg_in = nc.dram_tensor("wgate2", [2, 128, 2, 1], f16, kind="ExternalInput")
    we1_in = nc.dram_tensor("we1", [NE, 128, 2, D], f16, kind="ExternalInput")
    be1_in = nc.dram_tensor("be1", [NE, 128, 2], f32, kind="ExternalInput")
    we2_in = nc.dram_tensor("we2", [NE, 128, 2, OUT], f16, kind="ExternalInput")
    be2d_in = nc.dram_tensor("be2diff", [1, 2, 2, 128], f16,
                             kind="ExternalInput")  # (hi/lo, mc, col)
    be2b_in = nc.dram_tensor("be2base", [128, 2], f32, kind="ExternalInput")
    out_t = nc.dram_tensor("out", [OUT, NLOC], f32, kind="ExternalOutput")

    with tile.TileContext(nc) as tc:
        with (
            tc.tile_pool(name="const", bufs=1) as cpool,
            tc.tile_pool(name="work", bufs=2) as wpool,
            tc.tile_pool(name="slabp", bufs=4) as slabpool,
            tc.tile_pool(name="selp", bufs=4) as selpool,
            tc.tile_pool(name="stage", bufs=3) as stpool,
            tc.tile_pool(name="psum_sel", bufs=2, space="PSUM") as ps_sel,
            tc.tile_pool(name="psum_xf", bufs=2, space="PSUM") as ps_xf,
            tc.tile_pool(name="psum_misc", bufs=1, space="PSUM") as ps_misc,
            tc.tile_pool(name="dram", bufs=1, space="DRAM") as dpool,
            tc.tile_pool(name="dramsh", bufs=1, space="DRAM") as shpool,
        ):
            # constants / weights
            ident = cpool.tile([128, 128], f32)
            make_identity(nc, ident[:])
            ident16 = cpool.tile([128, 128], f16)
            nc.vector.tensor_copy(ident16[:], ident[:])
            iota_i = cpool.tile([128, WIN], dt.int32)
            nc.gpsimd.iota(iota_i[:], pattern=[[1, WIN]], base=0,
                           channel_multiplier=0)
            iota_f = cpool.tile([128, WIN], f32)
            nc.vector.tensor_copy(iota_f[:], iota_i[:])
            ones16 = cpool.tile([1, 128], f16)
            nc.vector.memset(ones16[:], 1.0)

            def load_const(t_in, shape, re=None, tag=None, cdt=f32):
                t = cpool.tile(shape, cdt, tag=tag)
                nc.sync.dma_start(t[:], t_in[:] if re is None else t_in[:].rearrange(re))
                return t

            # dense weights: [hilo, 128, 2, D] -> sbuf [128, hilo, 2, D]
            w_in_sb = load_const(w_in_in, [128, 2, 2, D], "h p k d -> p h k d",
                                 tag="w_in", cdt=f16)
            w_root_sb = load_const(w_root_in, [128, 2, 2, D],
                                   "h p k d -> p h k d", tag="w_root", cdt=f16)
            w_rel_sb = load_const(w_rel_in, [128, R, 2, D], "r p k d -> p r k d",
                                  tag="w_rel")
            b_in_sb = load_const(b_in_in, [128, 2], tag="b_in")
            b_rg_sb = load_const(b_rgcn_in, [128, 2], tag="b_rg")
            wgd_sb = load_const(wg_in, [128, 2, 2, 1], "h p k d -> p h k d",
                                tag="wgd", cdt=f16)
            we1_sb = load_const(we1_in, [128, NE, 2, D], "e p k d -> p e k d",
                                tag="we1", cdt=f16)
            be1_sb = load_const(be1_in, [128, NE, 2], "e p k -> p e k", tag="be1")
            we2_sb = load_const(we2_in, [128, NE, 2, OUT], "e p k d -> p e k d",
                                tag="we2", cdt=f16)
            be2d_sb = cpool.tile([1, 2, 2, 128], f16)
            nc.sync.dma_start(be2d_sb[:], be2d_in[:])
            be2b_sb = load_const(be2b_in, [128, 2], tag="be2b")

            seg_sb = cpool.tile([128, NSLOT], f32)
            nc.sync.dma_start(seg_sb[:], seg_in[:])
            idx_sb = cpool.tile([128, IDXC], dt.int16)
            nc.sync.dma_start(idx_sb[:], idx16_in[:])

            # DRAM staging: fp16 hi/lo pairs for inter-layer x
            xT1h = dpool.tile([128, 2, NLOC], f16)
            xT1l = dpool.tile([128, 2, NLOC], f16)
            xT2h = dpool.tile([128, 2, NLOC], f16)
            xT2l = dpool.tile([128, 2, NLOC], f16)
            xloc1 = dpool.tile([NLOC, TW], f16)
            xloc2 = dpool.tile([NLOC, TW], f16)
            xfulls = []  # [rep][layer][chunk]
            for _r in range(REPS):
                per_layer = []
                for li in (1, 2):
                    per_layer.append([
                        shpool.tile([NCORES * csz[k], TW], f16,
                                    addr_space="Shared",
                                    name="xf%d_%d_%d" % (li, _r, k))
                        for k in range(agc)
                    ])
                xfulls.append(per_layer)

            def win_sizes(w):
                wb = w * WIN
                return wb, min(WIN, NLOC - wb)

            def mm3(ps, w_sb, xh, xlo, mc, nw, start, stop):
                """3-term hi/lo dense matmul: out[mc] += W.T @ x (exact).

                w_sb: [128, hilo, 2, D] fp16; xh/xlo: [128, 2, nw] fp16.
                """
                for kc in range(2):
                    first = start and kc == 0
                    nc.tensor.matmul(
                        ps, w_sb[:, 0, kc, mc * 128:(mc + 1) * 128],
                        xh[:, kc, :nw], start=first, stop=False)
                    nc.tensor.matmul(
                        ps, w_sb[:, 0, kc, mc * 128:(mc + 1) * 128],
                        xlo[:, kc, :nw], start=False, stop=False)
                    last = stop and kc == 1
                    nc.tensor.matmul(
                        ps, w_sb[:, 1, kc, mc * 128:(mc + 1) * 128],
                        xh[:, kc, :nw], start=False, stop=last)

            def export_window(xh, xlo, wb, nw, xloc):
                # fp16 transpose (128, 2, nw) hi+lo -> node-major rows of xloc.
                # One packed PSUM tile = 1 bank; the two halves double-buffer
                # the per-128-node blocks manually.
                pst2 = ps_misc.tile([128, 2, TW], f16, space="PSUM",
                                    tag="misc16")
                nb = 0
                blk = 0
                while nb < nw:
                    bs = min(128, nw - nb)
                    stg = stpool.tile([128, TW], f16, tag="stage")
                    pst = pst2[:, blk % 2, :]
                    for mc in range(2):
                        nc.tensor.transpose(pst[:bs, mc * 128:(mc + 1) * 128],
                                            xh[:, mc, nb:nb + bs], ident16[:])
                        nc.tensor.transpose(pst[:bs, D + mc * 128:D + (mc + 1) * 128],
                                            xlo[:, mc, nb:nb + bs], ident16[:])
                    nc.scalar.activation(stg[:bs, :], pst[:bs, :], ACT.Copy)
                    nc.sync.dma_start(xloc[wb + nb: wb + nb + bs, :], stg[:bs, :])
                    nb += bs
                    blk += 1

            for _rep in range(REPS):
                xfull1, xfull2 = xfulls[_rep]
                ag_state1, ag_state2 = [0], [0]

                def ag_emit(xloc, xflist, rows_done, state):
                    while state[0] < agc and bounds[state[0] + 1] <= rows_done:
                        k = state[0]
                        if not SKIP_COLL:
                            nc.gpsimd.collective_compute(
                                "AllGather", mybir.AluOpType.bypass,
                                replica_groups=[list(range(NCORES))],
                                ins=[xloc[bounds[k]:bounds[k + 1], :].opt()],
                                outs=[xflist[k][:].opt()])
                        state[0] += 1

                # ------------ layer 0: x1 = selu(x_cat @ W_in + b_in) --------
                for w in range(NWIN):
                    wb, nw = win_sizes(w)
                    xwh = wpool.tile([128, 2, WIN], f16, tag="xwh")
                    xwl = wpool.tile([128, 2, WIN], f16, tag="xwl")
                    nc.sync.dma_start(xwh[:, :, :nw], xcat_h_in[:, :, wb:wb + nw])
                    nc.sync.dma_start(xwl[:, :, :nw], xcat_l_in[:, :, wb:wb + nw])
                    xoh = wpool.tile([128, 2, WIN], f16, tag="xoh")
                    xol = wpool.tile([128, 2, WIN], f16, tag="xol")
                    for mc in range(2):
                        ps = ps_xf.tile([128, WIN], f32, space="PSUM", tag="xf")
                        mm3(ps[:, :nw], w_in_sb, xwh, xwl, mc, nw,
                            start=True, stop=True)
                        pos = wpool.tile([128, WIN], f32, tag="selu_pos")
                        nc.vector.tensor_scalar(
                            out=pos[:, :nw], in0=ps[:, :nw],
                            scalar1=b_in_sb[:, mc:mc + 1], scalar2=0.0,
                            op0=Alu.add, op1=Alu.max)
                        neg = wpool.tile([128, WIN], f32, tag="selu_neg")
                        nc.vector.tensor_scalar(
                            out=neg[:, :nw], in0=ps[:, :nw],
                            scalar1=b_in_sb[:, mc:mc + 1], scalar2=0.0,
                            op0=Alu.add, op1=Alu.min)
                        e = wpool.tile([128, WIN], f32, tag="selu_e")
                        nc.scalar.activation(e[:, :nw], neg[:, :nw], ACT.Exp)
                        sa = SELU_SCALE * SELU_ALPHA
                        nc.vector.tensor_scalar(
                            out=e[:, :nw], in0=e[:, :nw], scalar1=sa, scalar2=sa,
                            op0=Alu.mult, op1=Alu.subtract)
                        nc.vector.tensor_scalar(
                            out=pos[:, :nw], in0=pos[:, :nw],
                            scalar1=SELU_SCALE, scalar2=None, op0=Alu.mult)
                        xo = wpool.tile([128, WIN], f32, tag="xo32")
                        nc.vector.tensor_tensor(
                            out=xo[:, :nw], in0=pos[:, :nw], in1=e[:, :nw],
                            op=Alu.add)
                        nc.scalar.activation(xoh[:, mc, :nw], xo[:, :nw],
                                             ACT.Copy)
                        nc.vector.tensor_tensor(
                            out=xol[:, mc, :nw], in0=xo[:, :nw],
                            in1=xoh[:, mc, :nw], op=Alu.subtract)
                    nc.sync.dma_start(xT1h[:, :, wb:wb + nw], xoh[:, :, :nw])
                    nc.sync.dma_start(xT1l[:, :, wb:wb + nw], xol[:, :, :nw])
                    export_window(xoh, xol, wb, nw, xloc1)
                    if AG_OVERLAP:
                        ag_emit(xloc1, xfull1, wb + nw, ag_state1)
                ag_emit(xloc1, xfull1, NLOC, ag_state1)

                # ------------ rgcn layers ------------
                def rgcn_layer(xfull, xTh_cur, xTl_cur, xTh_next, xTl_next,
                               xloc_next, li, moe_fn=None, xfull_next=None,
                               ag_state=None):
                    for w in range(NWIN):
                        wb, nw = win_sizes(w)
                        # load x and issue the root matmuls first: they only
                        # need xT, so the PE has work while gathers land
                        xwh = wpool.tile([128, 2, WIN], f16, tag="xwh")
                        xwl = wpool.tile([128, 2, WIN], f16, tag="xwl")
                        nc.sync.dma_start(xwh[:, :, :nw],
                                          xTh_cur[:, :, wb:wb + nw])
                        nc.sync.dma_start(xwl[:, :, :nw],
                                          xTl_cur[:, :, wb:wb + nw])
                        ps_mc = []
                        for mc in range(2):
                            ps = ps_xf.tile([128, WIN], f32, space="PSUM",
                                            tag="xf")
                            mm3(ps[:, :nw], w_root_sb, xwh, xwl, mc, nw,
                                start=True, stop=False)
                            ps_mc.append(ps)
                        s_tiles = {}
                        for r in range(R):
                            wi = r * NWIN + w
                            cursor = plan["win_slot_start"][wi]
                            kcounts = plan["win_slot_chunks"][wi]
                            if SLOT_CAP is not None:
                                kcounts = [min(kcounts[0], SLOT_CAP)] + \
                                    [0] * (len(kcounts) - 1)
                            nslots_w = sum(kcounts)
                            slab = slabpool.tile([128, MAX_SLOTS, TW], f16,
                                                 tag="slab")
                            off = 0
                            for k in range(agc):
                                ck, ni = plan["win_idx_cols"][wi][k]
                                nsl = kcounts[k]
                                ni = min(ni, nsl * CHUNK)
                                if ni and not SKIP_GATHER:
                                    nc.gpsimd.dma_gather(
                                        out_ap=slab[:, off:off + nsl, :],
                                        in_ap=xfull[k][:],
                                        idxs_ap=idx_sb[:, ck:ck + ni // 16],
                                        num_idxs=ni, num_idxs_reg=ni,
                                        elem_size=TW, single_packet=False,
                                        queue_num=(r * agc + k) % N_SWDGE_Q)
                                off += nsl
                            ps0 = ps_sel.tile([128, WIN], f32, space="PSUM",
                                              tag="sel0")
                            ps1 = ps_sel.tile([128, WIN], f32, space="PSUM",
                                              tag="sel1")
                            for s in range(nslots_w):
                                sl = cursor + s
                                gi = int(slot_group[sl])
                                gb_in_win = groups[gi][2] - wb
                                gs = groups[gi][3]
                                if gs > N_GRID:
                                    sel = selpool.tile([128, WIN], f16,
                                                       tag="sel_ovf")
                                else:
                                    sel = selpool.tile([128, N_GRID], f16,
                                                       tag="sel")
                                nc.vector.tensor_scalar(
                                    out=sel[:, :gs], in0=iota_f[:, :gs],
                                    scalar1=seg_sb[:, sl:sl + 1], scalar2=None,
                                    op0=Alu.is_equal)
                                cols = slice(gb_in_win, gb_in_win + gs)
                                last = s == nslots_w - 1
                                nc.tensor.matmul(
                                    ps0[:, cols], slab[:, s, 0:128],
                                    sel[:, :gs], start=(s == 0), stop=False)
                                nc.tensor.matmul(
                                    ps0[:, cols], slab[:, s, 256:384],
                                    sel[:, :gs], start=False, stop=last)
                                nc.tensor.matmul(
                                    ps1[:, cols], slab[:, s, 128:256],
                                    sel[:, :gs], start=(s == 0), stop=False)
                                nc.tensor.matmul(
                                    ps1[:, cols], slab[:, s, 384:512],
                                    sel[:, :gs], start=False, stop=last)
                            # broadcast cnt_inv to 128 rows: two fp16 rank-1
                            # matmuls (hi+lo) accumulate the exact f32 value
                            ci16 = wpool.tile([1, 2, WIN], f16, tag="ci_row")
                            nc.sync.dma_start(ci16[:], cntinv_in[0, wi:wi + 1])
                            cib = ps_misc.tile([128, WIN], f32, space="PSUM",
                                               tag="misc")
                            nc.tensor.matmul(cib[:, :nw], ones16[:],
                                             ci16[0:1, 0, :nw],
                                             start=True, stop=False)
                            nc.tensor.matmul(cib[:, :nw], ones16[:],
                                             ci16[0:1, 1, :nw],
                                             start=False, stop=True)
                            ci = wpool.tile([128, WIN], f32, tag="cntinv")
                            nc.scalar.activation(ci[:, :nw], cib[:, :nw], ACT.Copy)
                            s0 = wpool.tile([128, WIN], f32, tag="s0_%d" % r)
                            s1 = wpool.tile([128, WIN], f32, tag="s1_%d" % r)
                            nc.vector.tensor_tensor(out=s0[:, :nw], in0=ps0[:, :nw],
                                                    in1=ci[:, :nw], op=Alu.mult)
                            nc.vector.tensor_tensor(out=s1[:, :nw], in0=ps1[:, :nw],
                                                    in1=ci[:, :nw], op=Alu.mult)
                            s_tiles[r] = (s0, s1)

                        xoh = wpool.tile([128, 2, WIN], f16, tag="xoh")
                        xol = wpool.tile([128, 2, WIN], f16, tag="xol")
                        for mc in range(2):
                            ps = ps_mc[mc]
                            for r in range(R):
                                for kc in range(2):
                                    st = s_tiles[r][kc]
                                    nc.tensor.matmul(
                                        ps[:, :nw],
                                        w_rel_sb[:, r, kc, mc * 128:(mc + 1) * 128],
                                        st[:, :nw],
                                        start=False, stop=(r == R - 1 and kc == 1))
                            # x = ps + b, split to fp16 hi/lo (Identity
                            # accepts an AP bias; Copy does not)
                            nc.scalar.activation(xoh[:, mc, :nw], ps[:, :nw],
                                                 ACT.Identity,
                                                 bias=b_rg_sb[:, mc:mc + 1])
                            xf = wpool.tile([128, WIN], f32, tag="xf32")
                            nc.vector.tensor_scalar(
                                out=xf[:, :nw], in0=ps[:, :nw],
                                scalar1=b_rg_sb[:, mc:mc + 1], scalar2=None,
                                op0=Alu.add)
                            nc.vector.tensor_tensor(
                                out=xol[:, mc, :nw], in0=xf[:, :nw],
                                in1=xoh[:, mc, :nw], op=Alu.subtract)
                        if xTh_next is not None:
                            nc.sync.dma_start(xTh_next[:, :, wb:wb + nw],
                                              xoh[:, :, :nw])
                            nc.sync.dma_start(xTl_next[:, :, wb:wb + nw],
                                              xol[:, :, :nw])
                        if xloc_next is not None:
                            export_window(xoh, xol, wb, nw, xloc_next)
                            if xfull_next is not None and AG_OVERLAP:
                                ag_emit(xloc_next, xfull_next, wb + nw, ag_state)
                        if moe_fn is not None:
                            moe_fn(xoh, xol, wb, nw)
                    if xfull_next is not None:
                        ag_emit(xloc_next, xfull_next, NLOC, ag_state)

                # ------------ MoE (fused into layer-2 windows) ------------
                def moe_window(xh, xlo, wb, nw):
                    psl = ps_misc.tile([128, WIN], f32, space="PSUM", tag="misc")
                    for kc in range(2):
                        nc.tensor.matmul(psl[:1, :nw], wgd_sb[:, 0, kc, :],
                                         xh[:, kc, :nw], start=(kc == 0),
                                         stop=False)
                        nc.tensor.matmul(psl[:1, :nw], wgd_sb[:, 0, kc, :],
                                         xlo[:, kc, :nw], start=False, stop=False)
                        nc.tensor.matmul(psl[:1, :nw], wgd_sb[:, 1, kc, :],
                                         xh[:, kc, :nw], start=False,
                                         stop=(kc == 1))
                    g_row = wpool.tile([1, WIN], f32, tag="grow")
                    nc.vector.tensor_scalar(out=g_row[:, :nw], in0=psl[:1, :nw],
                                            scalar1=0.0, scalar2=None,
                                            op0=Alu.is_ge)
                    g16 = wpool.tile([1, WIN], f16, tag="g16")
                    nc.scalar.activation(g16[:, :nw], g_row[:, :nw], ACT.Copy)
                    psb = ps_misc.tile([128, WIN], f32, space="PSUM", tag="misc")
                    nc.tensor.matmul(psb[:, :nw], ones16[:], g16[:, :nw],
                                     start=True, stop=True)
                    gb = wpool.tile([128, WIN], f32, tag="gb_sb")
                    nc.scalar.activation(gb[:, :nw], psb[:, :nw], ACT.Copy)
                    ginv = wpool.tile([128, WIN], f32, tag="ginv")
                    nc.vector.tensor_scalar(out=ginv[:, :nw], in0=gb[:, :nw],
                                            scalar1=-1.0, scalar2=1.0,
                                            op0=Alu.mult, op1=Alu.add)

                    h1g = {}
                    for e in range(NE):
                        for mc in range(2):
                            psh = ps_xf.tile([128, WIN], f32, space="PSUM",
                                             tag="xf")
                            for kc in range(2):
                                nc.tensor.matmul(
                                    psh[:, :nw],
                                    we1_sb[:, e, kc, mc * 128:(mc + 1) * 128],
                                    xh[:, kc, :nw],
                                    start=(kc == 0), stop=(kc == 1))
                            h = wpool.tile([128, WIN], f16,
                                           tag="h1_%d_%d" % (e, mc))
                            nc.scalar.activation(
                                h[:, :nw], psh[:, :nw], ACT.Lrelu,
                                bias=be1_sb[:, e, mc:mc + 1], alpha=NEG_SLOPE)
                            gt = gb if e == 0 else ginv
                            nc.vector.tensor_tensor(out=h[:, :nw], in0=h[:, :nw],
                                                    in1=gt[:, :nw], op=Alu.mult)
                            h1g[(e, mc)] = h
                    for mc in range(2):
                        psy = ps_xf.tile([128, WIN], f32, space="PSUM", tag="xf")
                        first = True
                        for e in range(NE):
                            for kc in range(2):
                                nc.tensor.matmul(
                                    psy[:, :nw],
                                    we2_sb[:, e, kc, mc * 128:(mc + 1) * 128],
                                    h1g[(e, kc)][:, :nw],
                                    start=first, stop=False)
                                first = False
                        # + (be2[0]-be2[1]) (x) g  via hi/lo rank-1 fp16
                        nc.tensor.matmul(psy[:, :nw], be2d_sb[0:1, 0, mc, :],
                                         g16[:, :nw], start=False, stop=False)
                        nc.tensor.matmul(psy[:, :nw], be2d_sb[0:1, 1, mc, :],
                                         g16[:, :nw], start=False, stop=True)
                        yt = wpool.tile([128, WIN], f32, tag="yt")
                        nc.scalar.activation(yt[:, :nw], psy[:, :nw],
                                             ACT.Identity,
                                             bias=be2b_sb[:, mc:mc + 1])
                        nc.sync.dma_start(out_t[mc * 128:(mc + 1) * 128,
                                                wb:wb + nw],
                                          yt[:, :nw])

                rgcn_layer(xfull1, xT1h, xT1l, xT2h, xT2l, xloc2, 1,
                           xfull_next=xfull2, ag_state=ag_state2)
                rgcn_layer(xfull2, xT2h, xT2l, None, None, None, 2,
                           moe_fn=moe_window)

    nc.compile()
    return nc


# ----------------------------------------------------------------------------
# entry point
# ----------------------------------------------------------------------------

def _hilo(a):
    a = np.asarray(a, np.float32)
    hi = a.astype(np.float16)
    lo = (a - hi.astype(np.float32)).astype(np.float16)
    return hi, lo


def kernel(des, tweet, num_prop, cat_prop, edge_index, edge_type,
           W_in, b_in, W_rel, W_root, b_rgcn, w_gate, We1, be1, We2, be2):
    from concourse.bass_utils import run_bass_kernel_spmd

    x_cat = np.concatenate(
        [np.asarray(des), np.asarray(tweet), np.asarray(num_prop),
         np.asarray(cat_prop)], axis=1).astype(np.float32)

    plan, data = build_plan(np.asarray(edge_index), np.asarray(edge_type))
    nc = build_nc(plan)

    def wmat(w):
        w = np.asarray(w, np.float32)
        return np.ascontiguousarray(w.reshape(2, 128, w.shape[1]).transpose(1, 0, 2))

    def wmat2(w):
        return np.stack(_hilo(wmat(w)))  # (2=hi/lo, 128, 2, D)

    def bvec(b):
        return np.ascontiguousarray(np.asarray(b, np.float32).reshape(2, 128).T)

    w_rel_h = np.stack([wmat(np.asarray(W_rel)[r]) for r in range(R)])
    we1_h = np.stack([wmat(np.asarray(We1)[e]).astype(np.float16)
                      for e in range(NE)])
    be1_h = np.stack([bvec(np.asarray(be1)[e]) for e in range(NE)])
    we2_h = np.stack([wmat(np.asarray(We2)[e]).astype(np.float16)
                      for e in range(NE)])
    be2 = np.asarray(be2, np.float32)
    be2d_hi, be2d_lo = _hilo((be2[0] - be2[1]).reshape(1, 2, 128))
    be2d = np.stack([be2d_hi, be2d_lo], axis=1)  # (1, 2hilo, 2mc, 128)
    be2b = bvec(be2[1])

    wg = np.asarray(w_gate, np.float32)
    wgd2 = np.stack(_hilo(wmat(wg[:, 0:1] - wg[:, 1:2])))

    in_maps = []
    for c in range(NCORES):
        xc = x_cat[c * NLOC:(c + 1) * NLOC]
        xcatT = np.ascontiguousarray(xc.T.reshape(2, 128, NLOC).transpose(1, 0, 2))
        xch, xcl = _hilo(xcatT)
        in_maps.append({
            "xcath": xch,
            "xcatl": xcl,
            "idx16": data["idx16"][c],
            "segloc": data["seg_all"][c],
            "cntinv16": data["cntinv16"][c][None],
            "w_in2": wmat2(W_in), "w_root2": wmat2(W_root), "w_rel": w_rel_h,
            "b_in": bvec(b_in), "b_rgcn": bvec(b_rgcn),
            "wgate2": wgd2,
            "we1": we1_h, "be1": be1_h,
            "we2": we2_h, "be2diff": be2d, "be2base": be2b,
        })

    res = run_bass_kernel_spmd(nc, in_maps, core_ids=list(range(NCORES)))
    global last_nc, last_in_maps
    last_nc, last_in_maps = nc, in_maps
    y = np.concatenate([res.results[c]["out"].T for c in range(NCORES)], axis=0)
    return y.astype(np.float32)


last_nc = None
last_in_maps = None
